# revision 4
# baseline (speedup 1.0000x reference)
"""GCN encoder (2x GCNConv + LN/ReLU + sigmoid head) as a Bass/Trainium2 SPMD kernel.

Strategy (per sharding hint): destinations sharded 6250 nodes/core across 8
cores, edges partitioned by destination; per-core aggregation is done with
dma_gather (source-row gather) + one-hot scatter matmuls accumulating into
PSUM per 128-node destination window.  Two launches: layer 1 (+dense, LN,
heads) and layer 2, with the inter-layer halo exchange (full g2 table) done
on the host between launches.
"""

import os
import sys

for _p in ("/opt/trn_rl_repo", "/opt/pypackages"):
    if _p not in sys.path:
        sys.path.insert(0, _p)

import numpy as np
import ml_dtypes

import concourse.bass as bass
import concourse.mybir as mybir
import concourse.tile as tile
from concourse import bacc
from concourse.bass_utils import run_bass_kernel_spmd
from concourse.masks import make_identity

# ---- problem constants (hardcoded per task contract) ----
N = 50000
IN_C = 128
OUT_C = 64
NCORES = 8
SHARD = N // NCORES          # 6250
P = 128
NWIN = (SHARD + P - 1) // P  # 49 windows/shard; last window 106 nodes
LASTW = SHARD - (NWIN - 1) * P
HALF = 32768                 # int16 gather split
GW = 4                       # windows per gather group
NGRP = (NWIN + GW - 1) // GW # 13 groups (last group 1 window)
NTILE = (N + P - 1) // P     # 391 dense node tiles
NPAD = NTILE * P             # 50048 padded rows of the gather tables
DENSE_CHUNK = 16             # node tiles per dense xT chunk (2048 nodes)
EPS = 1e-5

DEBUG_PHASE = os.environ.get("KDBG", "full")  # dense | agg | full
TRACE = False                # test.py sets True to collect HW exec times
LAST_EXEC_NS = []
_PROF_SHIM = False


def _install_prof_shim():
    global _PROF_SHIM
    if _PROF_SHIM:
        return
    try:
        import antenv.axon_hooks  # noqa: F401
    except ImportError:
        import types
        from trn_agent_boot.trn_boot import _ntff_profile_via_ctypes
        hook = _ntff_profile_via_ctypes('/opt/axon/libaxon_pjrt.so')
        mod = types.ModuleType('antenv.axon_hooks')
        mod._hook = hook
        mod.get_axon_ntff_profile_hook = lambda: mod._hook
        mod.set_axon_ntff_profile_hook = lambda h: setattr(mod, '_hook', h)
        sys.modules['antenv.axon_hooks'] = mod
        import antenv
        antenv.axon_hooks = mod
    _PROF_SHIM = True


# --------------------------------------------------------------------------
# host-side schedule construction
# --------------------------------------------------------------------------

def build_schedule(edge_src, edge_dst):
    """Partition edges (+self loops) by destination into a chunk schedule that
    is structurally identical across all 8 cores (counts are maxed over cores,
    shorter cores padded with null edges: idx=0, woff=-1).

    Returns (sched, percore) where
      sched: dict with compile-time constants shared by all cores
      percore[r]: dict with idx16 [128, ncol], dstoff [128, nch] arrays
    """
    src = np.concatenate([edge_src, np.arange(N, dtype=np.int64)])
    dst = np.concatenate([edge_dst, np.arange(N, dtype=np.int64)])

    shard = dst // SHARD
    within = dst % SHARD
    win = within // P
    woff = within % P
    flag = (src >= HALF).astype(np.int64)

    # group edges by (shard, window, flag)
    key = (shard * NWIN + win) * 2 + flag
    order = np.argsort(key, kind="stable")
    skey = key[order]
    ssrc = src[order]
    swoff = woff[order]

    nkeys = NCORES * NWIN * 2
    counts = np.bincount(skey, minlength=nkeys).reshape(NCORES, NWIN, 2)
    starts = np.zeros(nkeys + 1, np.int64)
    np.cumsum(counts.reshape(-1), out=starts[1:])

    # uniform chunk counts: max over cores per (window, flag)
    chunks_rwf = -(-counts // P)                       # ceil
    kch = chunks_rwf.max(axis=0)                       # [NWIN, 2]
    klo, khi = kch[:, 0], kch[:, 1]
    assert klo.min() >= 1 and khi.min() >= 1

    # group layout: for each group, lo chunks of its windows then hi chunks
    grp_windows = [list(range(g * GW, min((g + 1) * GW, NWIN))) for g in range(NGRP)]
    grp_lo = [int(sum(klo[w] for w in ws)) for ws in grp_windows]
    grp_hi = [int(sum(khi[w] for w in ws)) for ws in grp_windows]
    grp_nch = [lo + hi for lo, hi in zip(grp_lo, grp_hi)]
    nch_total = int(sum(grp_nch))

    # global chunk index of each (window, flag) span + per-group call info
    chunk_off = {}
    calls = []   # (grp, flag, chunk_off_global, nchunks)
    off = 0
    for g, ws in enumerate(grp_windows):
        calls.append((g, 0, off, grp_lo[g]))
        for w in ws:
            chunk_off[(w, 0)] = off
            off += int(klo[w])
        calls.append((g, 1, off, grp_hi[g]))
        for w in ws:
            chunk_off[(w, 1)] = off
            off += int(khi[w])
    assert off == nch_total

    # per-core arrays
    percore = []
    for r in range(NCORES):
        idx_all = np.zeros(nch_total * P, np.int64)       # logical edge idx
        dstoff = np.full((P, nch_total), -1.0, np.float32)
        for w in range(NWIN):
            for f in (0, 1):
                k = starts[(r * NWIN + w) * 2 + f]
                e = starts[(r * NWIN + w) * 2 + f + 1]
                cnt = e - k
                co = chunk_off[(w, f)]
                base = HALF if f else 0
                span = idx_all[co * P: co * P + cnt]
                span[:] = ssrc[k:e] - base
                dcol = dstoff[:, co:co + int(kch[w, f])]
                flat = np.full(int(kch[w, f]) * P, -1.0, np.float32)
                flat[:cnt] = swoff[k:e]
                dcol[:] = flat.reshape(-1, P).T
        # wrap idx per gather call: position i -> [i % 16, i // 16], x8 replicate
        ncol = nch_total * (P // 16)
        idx16 = np.zeros((16, ncol), np.int16)
        for (_, _, co, nc_) in calls:
            if nc_ == 0:
                continue
            span = idx_all[co * P:(co + nc_) * P].astype(np.int16)
            blk = span.reshape(-1, 16).T                  # [16, n/16]
            idx16[:, co * 8:(co + nc_) * 8] = blk
        percore.append({
            "idx16": np.ascontiguousarray(np.tile(idx16, (8, 1))),
            "dstoff": dstoff,
        })

    sched = {
        "klo": klo, "khi": khi,
        "grp_windows": grp_windows,
        "grp_lo": grp_lo, "grp_hi": grp_hi, "grp_nch": grp_nch,
        "nch_total": nch_total,
        "chunk_off": chunk_off,
        "calls": calls,
        "max_nch": max(grp_nch),
    }
    return sched, percore


# --------------------------------------------------------------------------
# kernel builders
# --------------------------------------------------------------------------

def _emit_aggregation(nc, tc, pool, cpool, pp, sched, g_dram, idx_t, dstoff_t,
                      iota_f, window_epilogue, dep_insts):
    """Shared per-layer aggregation: for each group, 2 gathers + one-hot
    matmuls accumulating per-window PSUM, then window_epilogue(w, psum_tile).
    dep_insts: instructions every gather must wait for (g table writes)."""
    lo_tab = g_dram[0:HALF, :]
    hi_tab = g_dram[HALF:NPAD, :]
    calls = sched["calls"]
    for g, ws in enumerate(sched["grp_windows"]):
        nch_g = sched["grp_nch"][g]
        msgs = pool.tile([P, sched["max_nch"], P], mybir.dt.bfloat16, tag="msgs")
        # two gather calls for this group
        MAXCH = 8   # 1024 idxs/call: SWDGE descriptor-ring capacity
        for (gg, f, co, nc_) in calls:
            if gg != g or nc_ == 0:
                continue
            for s0 in range(0, nc_, MAXCH):
                sco = co + s0
                snc = min(MAXCH, nc_ - s0)
                co0 = sco - sched["chunk_off"][(ws[0], 0)]  # slot within group
                gi = nc.gpsimd.dma_gather(
                    out_ap=msgs[:, co0:co0 + snc, :],
                    in_ap=(hi_tab if f else lo_tab),
                    idxs_ap=idx_t[:, sco * 8:(sco + snc) * 8],
                    num_idxs=snc * P,
                    num_idxs_reg=snc * P,
                    elem_size=P,
                )
                for d in dep_insts:
                    bass._add_dep_helper(gi.ins, d.ins, True,
                                         "gather after table write")
        base_off = sched["chunk_off"][(ws[0], 0)]
        for w in ws:
            nchw = int(sched["klo"][w] + sched["khi"][w])
            spans = [(sched["chunk_off"][(w, 0)], int(sched["klo"][w])),
                     (sched["chunk_off"][(w, 1)], int(sched["khi"][w]))]
            psum = pp.tile([P, OUT_C], mybir.dt.float32, tag="aggps")
            ci = 0
            for (co, k) in spans:
                for c in range(k):
                    gco = co + c            # global chunk id
                    lco = gco - base_off    # slot in group msgs tile
                    S = pool.tile([P, P], mybir.dt.bfloat16, tag="S")
                    nc.vector.tensor_scalar(
                        out=S[:], in0=iota_f[:],
                        scalar1=dstoff_t[:, gco:gco + 1], scalar2=None,
                        op0=mybir.AluOpType.is_equal,
                    )
                    nc.tensor.matmul(
                        out=psum[:], lhsT=S[:], rhs=msgs[:, lco, 0:OUT_C],
                        start=(ci == 0), stop=(ci == nchw - 1),
                    )
                    ci += 1
            window_epilogue(w, psum)


def build_launch_a(sched):
    nc = bacc.Bacc("TRN2", target_bir_lowering=False, debug=False,
                   num_devices=NCORES)
    dt = mybir.dt
    xT_d = nc.dram_tensor("xT", [IN_C, N], dt.float32, kind="ExternalInput")
    W1_d = nc.dram_tensor("W1", [IN_C, OUT_C], dt.float32, kind="ExternalInput")
    W2_d = nc.dram_tensor("W2", [OUT_C, OUT_C], dt.float32, kind="ExternalInput")
    W3_d = nc.dram_tensor("W3", [OUT_C, 6], dt.float32, kind="ExternalInput")
    vecs_d = nc.dram_tensor("vecs", [P, 4 * OUT_C], dt.float32, kind="ExternalInput")
    dinv_t_d = nc.dram_tensor("dinv_t", [P, NTILE], dt.float32, kind="ExternalInput")
    dinv_w_d = nc.dram_tensor("dinv_w", [P, NWIN], dt.float32, kind="ExternalInput")
    ncol = sched["nch_total"] * 8
    idx_d = nc.dram_tensor("idx16", [P, ncol], dt.int16, kind="ExternalInput")
    dstoff_d = nc.dram_tensor("dstoff", [P, sched["nch_total"]], dt.float32,
                              kind="ExternalInput")
    g1_d = nc.dram_tensor("g1buf", [NPAD, P], dt.bfloat16)      # internal
    g2s_d = nc.dram_tensor("g2s", [NWIN * P, OUT_C], dt.bfloat16,
                           kind="ExternalOutput")
    out2_d = nc.dram_tensor("out2s", [NWIN * P, 6], dt.float32,
                            kind="ExternalOutput")

    with tile.TileContext(nc) as tc:
        with (
            tc.tile_pool(name="const", bufs=1) as cpool,
            tc.tile_pool(name="sbuf", bufs=2) as pool,
            tc.tile_pool(name="spool4", bufs=6) as spool,
            tc.tile_pool(name="psum", bufs=2, space="PSUM") as pp,
        ):
            # ---- constants ----
            W1_t = cpool.tile([IN_C, OUT_C], dt.float32)
            nc.sync.dma_start(out=W1_t[:], in_=W1_d[:])
            W2_t = cpool.tile([OUT_C, OUT_C], dt.float32)
            nc.sync.dma_start(out=W2_t[:], in_=W2_d[:])
            W3_t = cpool.tile([OUT_C, 6], dt.float32)
            nc.sync.dma_start(out=W3_t[:], in_=W3_d[:])
            vecs = cpool.tile([P, 4 * OUT_C], dt.float32)
            nc.sync.dma_start(out=vecs[:], in_=vecs_d[:])
            dinv_t = cpool.tile([P, NTILE], dt.float32)
            nc.sync.dma_start(out=dinv_t[:], in_=dinv_t_d[:])
            dinv_w = cpool.tile([P, NWIN], dt.float32)
            nc.sync.dma_start(out=dinv_w[:], in_=dinv_w_d[:])
            idx_t = cpool.tile([P, ncol], dt.int16)
            nc.sync.dma_start(out=idx_t[:], in_=idx_d[:])
            dstoff_t = cpool.tile([P, sched["nch_total"]], dt.float32)
            nc.sync.dma_start(out=dstoff_t[:], in_=dstoff_d[:])
            ident = cpool.tile([P, P], dt.float32)
            make_identity(nc, ident[:])
            eps_t = cpool.tile([P, 1], dt.float32)
            nc.gpsimd.memset(eps_t[:], EPS)
            invD_t = cpool.tile([P, 1], dt.float32)
            nc.gpsimd.memset(invD_t[:], 1.0 / OUT_C)
            iota_i = cpool.tile([P, P], dt.int32)
            nc.gpsimd.iota(iota_i[:], pattern=[[1, P]], base=0,
                           channel_multiplier=0)
            iota_f = cpool.tile([P, P], dt.float32)
            nc.vector.tensor_copy(out=iota_f[:], in_=iota_i[:])

            # ---- phase 1: dense g1 = dinv * (x @ W1), bf16, 256B rows ----
            g1_writes = []
            nchunks = (NTILE + DENSE_CHUNK - 1) // DENSE_CHUNK
            for c in range(nchunks):
                t0 = c * DENSE_CHUNK
                nt = min(DENSE_CHUNK, NTILE - t0)
                n0 = t0 * P
                nn = nt * P
                xc = pool.tile([IN_C, DENSE_CHUNK * P], dt.float32, tag="xc")
                nv = min(nn, N - n0)         # valid source columns
                if nv < nn:
                    nc.vector.memset(xc[:, 0:nn], 0.0)
                nc.sync.dma_start(out=xc[:, 0:nv], in_=xT_d[:, n0:n0 + nv])
                stage = pool.tile([P, DENSE_CHUNK, P], dt.bfloat16, tag="stage")
                nc.vector.memset(stage[:], 0.0)
                for s in range(nt):
                    ps = pp.tile([P, OUT_C], dt.float32, tag="dps")
                    nc.tensor.matmul(out=ps[:], lhsT=xc[:, s * P:(s + 1) * P],
                                     rhs=W1_t[:], start=True, stop=True)
                    nc.vector.tensor_scalar(
                        out=stage[:, s, 0:OUT_C], in0=ps[:],
                        scalar1=dinv_t[:, t0 + s:t0 + s + 1], scalar2=None,
                        op0=mybir.AluOpType.mult)
                wi = nc.sync.dma_start(
                    out=g1_d[n0:n0 + nn, :].rearrange("(s p) d -> p s d", p=P),
                    in_=stage[:, 0:nt, :])
                g1_writes.append(wi)

            # ---- output staging ----
            g2stage = cpool.tile([P, NWIN, OUT_C], dt.bfloat16)
            o2stage = cpool.tile([P, NWIN, 6], dt.float32)

            # ---- phase 2: aggregation + epilogue ----
            def epilogue(w, psum):
                tt = spool.tile([P, OUT_C], dt.float32, tag="tt")
                nc.vector.tensor_scalar(out=tt[:], in0=psum[:],
                                        scalar1=dinv_w[:, w:w + 1], scalar2=None,
                                        op0=mybir.AluOpType.mult)
                nc.vector.tensor_tensor(out=tt[:], in0=tt[:],
                                        in1=vecs[:, 0:OUT_C],
                                        op=mybir.AluOpType.add)
                mu = spool.tile([P, 1], dt.float32, tag="mu")
                nc.vector.tensor_reduce(out=mu[:], in_=tt[:],
                                        axis=mybir.AxisListType.X,
                                        op=mybir.AluOpType.add)
                nc.vector.tensor_scalar(out=mu[:], in0=mu[:], scalar1=1.0 / OUT_C,
                                        scalar2=None, op0=mybir.AluOpType.mult)
                nc.vector.tensor_scalar(out=tt[:], in0=tt[:], scalar1=mu[:],
                                        scalar2=None,
                                        op0=mybir.AluOpType.subtract)
                var = spool.tile([P, 1], dt.float32, tag="var")
                sq = spool.tile([P, OUT_C], dt.float32, tag="sq")
                nc.scalar.activation(out=sq[:], in_=tt[:],
                                     func=mybir.ActivationFunctionType.Square,
                                     accum_out=var[:])
                std = spool.tile([P, 1], dt.float32, tag="std")
                nc.scalar.activation(out=std[:], in_=var[:],
                                     func=mybir.ActivationFunctionType.Sqrt,
                                     scale=invD_t[:], bias=eps_t[:])
                rstd = spool.tile([P, 1], dt.float32, tag="rstd")
                nc.vector.reciprocal(out=rstd[:], in_=std[:])
                hh = spool.tile([P, OUT_C], dt.float32, tag="hh")
                nc.vector.tensor_scalar(out=hh[:], in0=tt[:], scalar1=rstd[:],
                                        scalar2=None, op0=mybir.AluOpType.mult)
                nc.vector.tensor_tensor(out=hh[:], in0=hh[:],
                                        in1=vecs[:, OUT_C:2 * OUT_C],
                                        op=mybir.AluOpType.mult)
                nc.vector.tensor_tensor(out=hh[:], in0=hh[:],
                                        in1=vecs[:, 2 * OUT_C:3 * OUT_C],
                                        op=mybir.AluOpType.add)
                nc.scalar.activation(out=hh[:], in_=hh[:],
                                     func=mybir.ActivationFunctionType.Relu)
                # hT via PE transpose
                psT = pp.tile([OUT_C, P], dt.float32, tag="psT")
                nc.tensor.transpose(out=psT[:], in_=hh[:], identity=ident[:])
                hT = spool.tile([OUT_C, P], dt.float32, tag="hT")
                nc.vector.tensor_copy(out=hT[:], in_=psT[:])
                # g2 = dinv * (h @ W2); out2 = sigmoid(h @ W3 + b3)
                pst23 = pp.tile([P, OUT_C + 6], dt.float32, tag="pst23")
                nc.tensor.matmul(out=pst23[:, 0:OUT_C], lhsT=hT[:], rhs=W2_t[:],
                                 start=True, stop=True)
                nc.tensor.matmul(out=pst23[:, OUT_C:OUT_C + 6], lhsT=hT[:],
                                 rhs=W3_t[:], start=True, stop=True)
                nc.vector.tensor_scalar(out=g2stage[:, w, :],
                                        in0=pst23[:, 0:OUT_C],
                                        scalar1=dinv_w[:, w:w + 1], scalar2=None,
                                        op0=mybir.AluOpType.mult)
                o2 = spool.tile([P, 6], dt.float32, tag="o2")
                nc.vector.tensor_tensor(out=o2[:], in0=pst23[:, OUT_C:OUT_C + 6],
                                        in1=vecs[:, 3 * OUT_C:3 * OUT_C + 6],
                                        op=mybir.AluOpType.add)
                nc.scalar.activation(out=o2stage[:, w, :], in_=o2[:],
                                     func=mybir.ActivationFunctionType.Sigmoid)

            if DEBUG_PHASE == "dense":
                nc.vector.memset(g2stage[:], 0.0)
                nc.vector.memset(o2stage[:], 0.0)
            elif DEBUG_PHASE == "agg":
                nc.vector.memset(o2stage[:], 0.0)
                def epilogue_lite(w, psum):
                    nc.vector.tensor_copy(out=g2stage[:, w, :], in_=psum[:])
                _emit_aggregation(nc, tc, pool, cpool, pp, sched, g1_d, idx_t,
                                  dstoff_t, iota_f, epilogue_lite, g1_writes)
            else:
                _emit_aggregation(nc, tc, pool, cpool, pp, sched, g1_d, idx_t,
                                  dstoff_t, iota_f, epilogue, g1_writes)

            nc.sync.dma_start(
                out=g2s_d[:].rearrange("(w p) d -> p w d", p=P),
                in_=g2stage[:])
            nc.sync.dma_start(
                out=out2_d[:].rearrange("(w p) d -> p w d", p=P),
                in_=o2stage[:])
    nc.compile()
    return nc


def build_launch_b(sched):
    nc = bacc.Bacc("TRN2", target_bir_lowering=False, debug=False,
                   num_devices=NCORES)
    dt = mybir.dt
    g2_d = nc.dram_tensor("g2full", [NPAD, P], dt.bfloat16, kind="ExternalInput")
    dinv_w_d = nc.dram_tensor("dinv_w", [P, NWIN], dt.float32, kind="ExternalInput")
    b2_d = nc.dram_tensor("b2vec", [P, OUT_C], dt.float32, kind="ExternalInput")
    ncol = sched["nch_total"] * 8
    idx_d = nc.dram_tensor("idx16", [P, ncol], dt.int16, kind="ExternalInput")
    dstoff_d = nc.dram_tensor("dstoff", [P, sched["nch_total"]], dt.float32,
                              kind="ExternalInput")
    out1_d = nc.dram_tensor("out1s", [NWIN * P, OUT_C], dt.float32,
                            kind="ExternalOutput")

    with tile.TileContext(nc) as tc:
        with (
            tc.tile_pool(name="const", bufs=1) as cpool,
            tc.tile_pool(name="sbuf", bufs=2) as pool,
            tc.tile_pool(name="spool4", bufs=6) as spool,
            tc.tile_pool(name="psum", bufs=2, space="PSUM") as pp,
        ):
            dinv_w = cpool.tile([P, NWIN], dt.float32)
            nc.sync.dma_start(out=dinv_w[:], in_=dinv_w_d[:])
            b2v = cpool.tile([P, OUT_C], dt.float32)
            nc.sync.dma_start(out=b2v[:], in_=b2_d[:])
            idx_t = cpool.tile([P, ncol], dt.int16)
            nc.sync.dma_start(out=idx_t[:], in_=idx_d[:])
            dstoff_t = cpool.tile([P, sched["nch_total"]], dt.float32)
            nc.sync.dma_start(out=dstoff_t[:], in_=dstoff_d[:])
            iota_i = cpool.tile([P, P], dt.int32)
            nc.gpsimd.iota(iota_i[:], pattern=[[1, P]], base=0,
                           channel_multiplier=0)
            iota_f = cpool.tile([P, P], dt.float32)
            nc.vector.tensor_copy(out=iota_f[:], in_=iota_i[:])

            o1stage = cpool.tile([P, NWIN, OUT_C], dt.float32)

            def epilogue(w, psum):
                t1 = spool.tile([P, OUT_C], dt.float32, tag="t1")
                nc.vector.tensor_scalar(out=t1[:], in0=psum[:],
                                        scalar1=dinv_w[:, w:w + 1], scalar2=None,
                                        op0=mybir.AluOpType.mult)
                nc.vector.tensor_tensor(out=o1stage[:, w, :], in0=t1[:],
                                        in1=b2v[:], op=mybir.AluOpType.add)

            _emit_aggregation(nc, tc, pool, cpool, pp, sched, g2_d, idx_t,
                              dstoff_t, iota_f, epilogue, [])

            nc.sync.dma_start(
                out=out1_d[:].rearrange("(w p) d -> p w d", p=P),
                in_=o1stage[:])
    nc.compile()
    return nc


# --------------------------------------------------------------------------
# entry point
# --------------------------------------------------------------------------

def kernel(x, edge_index, W1, b1, ln_w, ln_b, W2, b2, W3, b3):
    x = np.asarray(x, np.float32)
    edge_index = np.asarray(edge_index)
    W1 = np.asarray(W1, np.float32); b1 = np.asarray(b1, np.float32)
    ln_w = np.asarray(ln_w, np.float32); ln_b = np.asarray(ln_b, np.float32)
    W2 = np.asarray(W2, np.float32); b2 = np.asarray(b2, np.float32)
    W3 = np.asarray(W3, np.float32); b3 = np.asarray(b3, np.float32)

    if TRACE:
        _install_prof_shim()
    del LAST_EXEC_NS[:]

    esrc = edge_index[0].astype(np.int64)
    edst = edge_index[1].astype(np.int64)

    # degrees (incl. self loop) and dinv, host-side (index preprocessing)
    deg = (np.bincount(edst, minlength=N) + 1).astype(np.float32)
    dinv = (1.0 / np.sqrt(deg)).astype(np.float32)

    sched, percore = build_schedule(esrc, edst)

    # host-side input prep
    xT = np.ascontiguousarray(x.T)
    vecs = np.ascontiguousarray(np.tile(np.concatenate(
        [b1, ln_w, ln_b, np.pad(b3, (0, OUT_C - 6))]), (P, 1)).astype(np.float32))
    dinv_pad = np.concatenate([dinv, np.ones(NPAD - N, np.float32)])
    dinv_t = np.ascontiguousarray(dinv_pad.reshape(NTILE, P).T)
    dinv_w_cores = []
    for r in range(NCORES):
        dw = np.ones(NWIN * P, np.float32)
        dw[:SHARD] = dinv[r * SHARD:(r + 1) * SHARD]
        dinv_w_cores.append(np.ascontiguousarray(dw.reshape(NWIN, P).T))

    ncA = build_launch_a(sched)
    in_maps_a = []
    for r in range(NCORES):
        in_maps_a.append({
            "xT": xT, "W1": W1, "W2": W2, "W3": W3, "vecs": vecs,
            "dinv_t": dinv_t, "dinv_w": dinv_w_cores[r],
            "idx16": percore[r]["idx16"], "dstoff": percore[r]["dstoff"],
        })
    resA = run_bass_kernel_spmd(ncA, in_maps_a, core_ids=list(range(NCORES)),
                                trace=TRACE)
    if TRACE:
        LAST_EXEC_NS.append(resA.exec_time_ns)

    # assemble full g2 table [NPAD, 128] bf16 (padding cols/rows zero)
    g2full = np.zeros((NPAD, P), ml_dtypes.bfloat16)
    out2 = np.empty((N, 6), np.float32)
    for r in range(NCORES):
        g2s = resA.results[r]["g2s"][:SHARD]           # [6250, 64]
        g2full[r * SHARD:(r + 1) * SHARD, 0:OUT_C] = g2s
        out2[r * SHARD:(r + 1) * SHARD] = resA.results[r]["out2s"][:SHARD]

    b2v = np.ascontiguousarray(np.tile(b2, (P, 1)).astype(np.float32))
    ncB = build_launch_b(sched)
    in_maps_b = []
    for r in range(NCORES):
        in_maps_b.append({
            "g2full": g2full, "dinv_w": dinv_w_cores[r], "b2vec": b2v,
            "idx16": percore[r]["idx16"], "dstoff": percore[r]["dstoff"],
        })
    resB = run_bass_kernel_spmd(ncB, in_maps_b, core_ids=list(range(NCORES)),
                                trace=TRACE)
    if TRACE:
        LAST_EXEC_NS.append(resB.exec_time_ns)

    out1 = np.empty((N, OUT_C), np.float32)
    for r in range(NCORES):
        out1[r * SHARD:(r + 1) * SHARD] = resB.results[r]["out1s"][:SHARD]

    return (out1, out2)


# revision 8
# speedup vs baseline: 1.0551x; 1.0551x over previous
"""GCN encoder (2x GCNConv + LN/ReLU + sigmoid head) as a Bass/Trainium2 SPMD kernel.

Strategy (per sharding hint): destinations sharded 6250 nodes/core across 8
cores, edges partitioned by destination; per-core aggregation is done with
dma_gather (source-row gather) + one-hot scatter matmuls accumulating into
PSUM per 128-node destination window.  Two launches: layer 1 (+dense, LN,
heads) and layer 2, with the inter-layer halo exchange (full g2 table) done
on the host between launches.
"""

import os
import sys

for _p in ("/opt/trn_rl_repo", "/opt/pypackages"):
    if _p not in sys.path:
        sys.path.insert(0, _p)

import numpy as np
import ml_dtypes

import concourse.bass as bass
import concourse.mybir as mybir
import concourse.tile as tile
from concourse import bacc
from concourse.bass_utils import run_bass_kernel_spmd
from concourse.masks import make_identity

# ---- problem constants (hardcoded per task contract) ----
N = 50000
IN_C = 128
OUT_C = 64
NCORES = 8
SHARD = N // NCORES          # 6250
P = 128
NWIN = (SHARD + P - 1) // P  # 49 windows/shard; last window 106 nodes
LASTW = SHARD - (NWIN - 1) * P
HALF = 32768                 # int16 gather split
GW = 4                       # windows per gather group
NGRP = (NWIN + GW - 1) // GW # 13 groups (last group 1 window)
NTILE = (N + P - 1) // P     # 391 dense node tiles
NPAD = NTILE * P             # 50048 padded rows of the gather tables
DENSE_CHUNK = 16             # node tiles per dense xT chunk (2048 nodes)
EPS = 1e-5

DEBUG_PHASE = os.environ.get("KDBG", "full")  # dense | agg | full
TRACE = False                # test.py sets True to collect HW exec times
LAST_EXEC_NS = []
LAST_RESULTS = []            # BassKernelResults per launch when TRACE
_PROF_SHIM = False


def _install_prof_shim():
    global _PROF_SHIM
    if _PROF_SHIM:
        return
    try:
        import antenv.axon_hooks  # noqa: F401
    except ImportError:
        import types
        from trn_agent_boot.trn_boot import _ntff_profile_via_ctypes
        hook = _ntff_profile_via_ctypes('/opt/axon/libaxon_pjrt.so')
        mod = types.ModuleType('antenv.axon_hooks')
        mod._hook = hook
        mod.get_axon_ntff_profile_hook = lambda: mod._hook
        mod.set_axon_ntff_profile_hook = lambda h: setattr(mod, '_hook', h)
        sys.modules['antenv.axon_hooks'] = mod
        import antenv
        antenv.axon_hooks = mod
    _PROF_SHIM = True


# --------------------------------------------------------------------------
# host-side schedule construction
# --------------------------------------------------------------------------

def build_schedule(edge_src, edge_dst):
    """Partition edges (+self loops) by destination into a chunk schedule that
    is structurally identical across all 8 cores (counts are maxed over cores,
    shorter cores padded with null edges: idx=0, woff=-1).

    Returns (sched, percore) where
      sched: dict with compile-time constants shared by all cores
      percore[r]: dict with idx16 [128, ncol], dstoff [128, nch] arrays
    """
    src = np.concatenate([edge_src, np.arange(N, dtype=np.int64)])
    dst = np.concatenate([edge_dst, np.arange(N, dtype=np.int64)])

    shard = dst // SHARD
    within = dst % SHARD
    win = within // P
    woff = within % P
    flag = (src >= HALF).astype(np.int64)

    # group edges by (shard, window, flag)
    key = (shard * NWIN + win) * 2 + flag
    order = np.argsort(key, kind="stable")
    skey = key[order]
    ssrc = src[order]
    swoff = woff[order]

    nkeys = NCORES * NWIN * 2
    counts = np.bincount(skey, minlength=nkeys).reshape(NCORES, NWIN, 2)
    starts = np.zeros(nkeys + 1, np.int64)
    np.cumsum(counts.reshape(-1), out=starts[1:])

    # uniform chunk counts: max over cores per (window, flag)
    chunks_rwf = -(-counts // P)                       # ceil
    kch = chunks_rwf.max(axis=0)                       # [NWIN, 2]
    klo, khi = kch[:, 0], kch[:, 1]
    assert klo.min() >= 1 and khi.min() >= 1

    # group layout: for each group, lo chunks of its windows then hi chunks
    grp_windows = [list(range(g * GW, min((g + 1) * GW, NWIN))) for g in range(NGRP)]
    grp_lo = [int(sum(klo[w] for w in ws)) for ws in grp_windows]
    grp_hi = [int(sum(khi[w] for w in ws)) for ws in grp_windows]
    grp_nch = [lo + hi for lo, hi in zip(grp_lo, grp_hi)]
    nch_total = int(sum(grp_nch))

    # global chunk index of each (window, flag) span + per-group call info
    chunk_off = {}
    calls = []   # (grp, flag, chunk_off_global, nchunks)
    off = 0
    for g, ws in enumerate(grp_windows):
        calls.append((g, 0, off, grp_lo[g]))
        for w in ws:
            chunk_off[(w, 0)] = off
            off += int(klo[w])
        calls.append((g, 1, off, grp_hi[g]))
        for w in ws:
            chunk_off[(w, 1)] = off
            off += int(khi[w])
    assert off == nch_total

    # per-core arrays
    percore = []
    for r in range(NCORES):
        idx_all = np.zeros(nch_total * P, np.int64)       # logical edge idx
        dstoff = np.full((P, nch_total), -1.0, np.float32)
        for w in range(NWIN):
            for f in (0, 1):
                k = starts[(r * NWIN + w) * 2 + f]
                e = starts[(r * NWIN + w) * 2 + f + 1]
                cnt = e - k
                co = chunk_off[(w, f)]
                base = HALF if f else 0
                span = idx_all[co * P: co * P + cnt]
                span[:] = ssrc[k:e] - base
                dcol = dstoff[:, co:co + int(kch[w, f])]
                flat = np.full(int(kch[w, f]) * P, -1.0, np.float32)
                flat[:cnt] = swoff[k:e]
                dcol[:] = flat.reshape(-1, P).T
        # wrap idx per gather call: position i -> [i % 16, i // 16], x8 replicate
        ncol = nch_total * (P // 16)
        idx16 = np.zeros((16, ncol), np.int16)
        for (_, _, co, nc_) in calls:
            if nc_ == 0:
                continue
            span = idx_all[co * P:(co + nc_) * P].astype(np.int16)
            blk = span.reshape(-1, 16).T                  # [16, n/16]
            idx16[:, co * 8:(co + nc_) * 8] = blk
        percore.append({
            "idx16": np.ascontiguousarray(np.tile(idx16, (8, 1))),
            "dstoff": dstoff,
        })

    sched = {
        "klo": klo, "khi": khi,
        "grp_windows": grp_windows,
        "grp_lo": grp_lo, "grp_hi": grp_hi, "grp_nch": grp_nch,
        "nch_total": nch_total,
        "chunk_off": chunk_off,
        "calls": calls,
        "max_nch": max(grp_nch),
    }
    return sched, percore


# --------------------------------------------------------------------------
# kernel builders
# --------------------------------------------------------------------------

def _emit_aggregation(nc, tc, pool, cpool, pp, sched, g_dram, idx_t, dstoff_t,
                      iota8, window_epilogue, dep_insts):
    """Shared per-layer aggregation: for each group, 2 gathers + one-hot
    matmuls accumulating per-window PSUM, then window_epilogue(w, psum_tile).
    dep_insts: instructions every gather must wait for (g table writes)."""
    lo_tab = g_dram[0:HALF, :]
    hi_tab = g_dram[HALF:NPAD, :]
    calls = sched["calls"]
    MAXCH = 8   # 1024 idxs/call: SWDGE descriptor-ring capacity
    for g, ws in enumerate(sched["grp_windows"]):
        base_off = sched["chunk_off"][(ws[0], 0)]
        msgs = pool.tile([P, sched["max_nch"], P], mybir.dt.bfloat16, tag="msgs")
        S_grp = pool.tile([P, sched["max_nch"], P], mybir.dt.bfloat16, tag="Sgrp")
        for (gg, f, co, nc_) in calls:
            if gg != g or nc_ == 0:
                continue
            for s0 in range(0, nc_, MAXCH):
                sco = co + s0
                snc = min(MAXCH, nc_ - s0)
                co0 = sco - base_off                      # slot within group
                gi = nc.gpsimd.dma_gather(
                    out_ap=msgs[:, co0:co0 + snc, :],
                    in_ap=(hi_tab if f else lo_tab),
                    idxs_ap=idx_t[:, sco * 8:(sco + snc) * 8],
                    num_idxs=snc * P,
                    num_idxs_reg=snc * P,
                    elem_size=P,
                )
                for d in dep_insts:
                    bass._add_dep_helper(gi.ins, d.ins, True,
                                         "gather after table write")
                # one-hot scatter matrices for this call's chunks, one DVE op
                nc.vector.tensor_tensor(
                    out=S_grp[:, co0:co0 + snc, :],
                    in0=iota8[:, 0:snc, :],
                    in1=dstoff_t[:, sco:sco + snc].broadcast_to([P, snc, P]),
                    op=mybir.AluOpType.is_equal,
                )
        for w in ws:
            nchw = int(sched["klo"][w] + sched["khi"][w])
            spans = [(sched["chunk_off"][(w, 0)], int(sched["klo"][w])),
                     (sched["chunk_off"][(w, 1)], int(sched["khi"][w]))]
            psum = pp.tile([P, OUT_C], mybir.dt.float32, tag="aggps")
            ci = 0
            for (co, k) in spans:
                for c in range(k):
                    lco = co + c - base_off  # slot in group msgs/S tiles
                    nc.tensor.matmul(
                        out=psum[:], lhsT=S_grp[:, lco, :],
                        rhs=msgs[:, lco, 0:OUT_C],
                        start=(ci == 0), stop=(ci == nchw - 1),
                    )
                    ci += 1
            window_epilogue(w, psum)


def build_launch_a(sched):
    nc = bacc.Bacc("TRN2", target_bir_lowering=False, debug=False,
                   num_devices=NCORES)
    dt = mybir.dt
    xT_d = nc.dram_tensor("xT", [IN_C, N], dt.float32, kind="ExternalInput")
    W1_d = nc.dram_tensor("W1", [IN_C, OUT_C], dt.float32, kind="ExternalInput")
    W2_d = nc.dram_tensor("W2", [OUT_C, OUT_C], dt.float32, kind="ExternalInput")
    W3_d = nc.dram_tensor("W3", [OUT_C, 6], dt.float32, kind="ExternalInput")
    vecs_d = nc.dram_tensor("vecs", [P, 4 * OUT_C], dt.float32, kind="ExternalInput")
    dinv_t_d = nc.dram_tensor("dinv_t", [P, NTILE], dt.float32, kind="ExternalInput")
    dinv_w_d = nc.dram_tensor("dinv_w", [P, NWIN], dt.float32, kind="ExternalInput")
    ncol = sched["nch_total"] * 8
    idx_d = nc.dram_tensor("idx16", [P, ncol], dt.int16, kind="ExternalInput")
    dstoff_d = nc.dram_tensor("dstoff", [P, sched["nch_total"]], dt.float32,
                              kind="ExternalInput")
    g1_d = nc.dram_tensor("g1buf", [NPAD, P], dt.bfloat16)      # internal
    g2s_d = nc.dram_tensor("g2s", [NWIN * P, OUT_C], dt.bfloat16,
                           kind="ExternalOutput")
    out2_d = nc.dram_tensor("out2s", [NWIN * P, 6], dt.float32,
                            kind="ExternalOutput")

    with tile.TileContext(nc) as tc:
        with (
            tc.tile_pool(name="const", bufs=1) as cpool,
            tc.tile_pool(name="sbuf", bufs=2) as pool,
            tc.tile_pool(name="spool4", bufs=6) as spool,
            tc.tile_pool(name="psum", bufs=2, space="PSUM") as pp,
        ):
            # ---- constants ----
            W1_t = cpool.tile([IN_C, OUT_C], dt.float32)
            nc.sync.dma_start(out=W1_t[:], in_=W1_d[:])
            W2_t = cpool.tile([OUT_C, OUT_C], dt.float32)
            nc.sync.dma_start(out=W2_t[:], in_=W2_d[:])
            W3_t = cpool.tile([OUT_C, 6], dt.float32)
            nc.sync.dma_start(out=W3_t[:], in_=W3_d[:])
            vecs = cpool.tile([P, 4 * OUT_C], dt.float32)
            nc.sync.dma_start(out=vecs[:], in_=vecs_d[:])
            dinv_t = cpool.tile([P, NTILE], dt.float32)
            nc.sync.dma_start(out=dinv_t[:], in_=dinv_t_d[:])
            dinv_w = cpool.tile([P, NWIN], dt.float32)
            nc.sync.dma_start(out=dinv_w[:], in_=dinv_w_d[:])
            idx_t = cpool.tile([P, ncol], dt.int16)
            nc.sync.dma_start(out=idx_t[:], in_=idx_d[:])
            dstoff_t = cpool.tile([P, sched["nch_total"]], dt.float32)
            nc.sync.dma_start(out=dstoff_t[:], in_=dstoff_d[:])
            ident = cpool.tile([P, P], dt.float32)
            make_identity(nc, ident[:])
            eps_t = cpool.tile([P, 1], dt.float32)
            nc.gpsimd.memset(eps_t[:], EPS)
            invD_t = cpool.tile([P, 1], dt.float32)
            nc.gpsimd.memset(invD_t[:], 1.0 / OUT_C)
            iota_i = cpool.tile([P, 8, P], dt.int32)
            nc.gpsimd.iota(iota_i[:], pattern=[[0, 8], [1, P]], base=0,
                           channel_multiplier=0)
            iota8 = cpool.tile([P, 8, P], dt.float32)
            nc.vector.tensor_copy(out=iota8[:], in_=iota_i[:])

            # ---- phase 1: dense g1 = dinv * (x @ W1), bf16, 256B rows ----
            g1_writes = []
            nchunks = (NTILE + DENSE_CHUNK - 1) // DENSE_CHUNK
            for c in range(nchunks):
                t0 = c * DENSE_CHUNK
                nt = min(DENSE_CHUNK, NTILE - t0)
                n0 = t0 * P
                nn = nt * P
                xc = pool.tile([IN_C, DENSE_CHUNK * P], dt.float32, tag="xc")
                nv = min(nn, N - n0)         # valid source columns
                if nv < nn:
                    nc.vector.memset(xc[:, 0:nn], 0.0)
                nc.sync.dma_start(out=xc[:, 0:nv], in_=xT_d[:, n0:n0 + nv])
                stage = pool.tile([P, DENSE_CHUNK, P], dt.bfloat16, tag="stage")
                nc.vector.memset(stage[:], 0.0)
                for q in range(0, nt, 4):
                    qn = min(4, nt - q)
                    ps4 = pp.tile([P, 4, OUT_C], dt.float32, tag="dps")
                    for s in range(qn):
                        nc.tensor.matmul(out=ps4[:, s, :],
                                         lhsT=xc[:, (q + s) * P:(q + s + 1) * P],
                                         rhs=W1_t[:], start=True, stop=True)
                    nc.vector.tensor_tensor(
                        out=stage[:, q:q + qn, 0:OUT_C], in0=ps4[:, 0:qn, :],
                        in1=dinv_t[:, t0 + q:t0 + q + qn]
                            .broadcast_to([P, qn, OUT_C]),
                        op=mybir.AluOpType.mult)
                wi = nc.sync.dma_start(
                    out=g1_d[n0:n0 + nn, :].rearrange("(s p) d -> p s d", p=P),
                    in_=stage[:, 0:nt, :])
                g1_writes.append(wi)

            # ---- output staging ----
            g2stage = cpool.tile([P, NWIN, OUT_C], dt.bfloat16)
            o2stage = cpool.tile([P, NWIN, 6], dt.float32)

            # ---- phase 2: aggregation + epilogue ----
            def epilogue(w, psum):
                tt = spool.tile([P, OUT_C], dt.float32, tag="tt")
                nc.vector.tensor_scalar(out=tt[:], in0=psum[:],
                                        scalar1=dinv_w[:, w:w + 1], scalar2=None,
                                        op0=mybir.AluOpType.mult)
                nc.vector.tensor_tensor(out=tt[:], in0=tt[:],
                                        in1=vecs[:, 0:OUT_C],
                                        op=mybir.AluOpType.add)
                mu = spool.tile([P, 1], dt.float32, tag="mu")
                nc.vector.tensor_reduce(out=mu[:], in_=tt[:],
                                        axis=mybir.AxisListType.X,
                                        op=mybir.AluOpType.add)
                nc.vector.tensor_scalar(out=mu[:], in0=mu[:], scalar1=1.0 / OUT_C,
                                        scalar2=None, op0=mybir.AluOpType.mult)
                nc.vector.tensor_scalar(out=tt[:], in0=tt[:], scalar1=mu[:],
                                        scalar2=None,
                                        op0=mybir.AluOpType.subtract)
                var = spool.tile([P, 1], dt.float32, tag="var")
                sq = spool.tile([P, OUT_C], dt.float32, tag="sq")
                if os.environ.get("KTTR", "0") == "1":
                    nc.vector.tensor_tensor_reduce(
                        out=sq[:], in0=tt[:], in1=tt[:], scale=1.0, scalar=0.0,
                        op0=mybir.AluOpType.mult, op1=mybir.AluOpType.add,
                        accum_out=var[:])
                else:
                    nc.scalar.activation(out=sq[:], in_=tt[:],
                                         func=mybir.ActivationFunctionType.Square,
                                         accum_out=var[:])
                std = spool.tile([P, 1], dt.float32, tag="std")
                nc.scalar.activation(out=std[:], in_=var[:],
                                     func=mybir.ActivationFunctionType.Sqrt,
                                     scale=invD_t[:], bias=eps_t[:])
                rstd = spool.tile([P, 1], dt.float32, tag="rstd")
                nc.vector.reciprocal(out=rstd[:], in_=std[:])
                hh = spool.tile([P, OUT_C], dt.float32, tag="hh")
                nc.vector.tensor_scalar(out=hh[:], in0=tt[:], scalar1=rstd[:],
                                        scalar2=None, op0=mybir.AluOpType.mult)
                nc.vector.tensor_tensor(out=hh[:], in0=hh[:],
                                        in1=vecs[:, OUT_C:2 * OUT_C],
                                        op=mybir.AluOpType.mult)
                nc.vector.tensor_tensor(out=hh[:], in0=hh[:],
                                        in1=vecs[:, 2 * OUT_C:3 * OUT_C],
                                        op=mybir.AluOpType.add)
                nc.scalar.activation(out=hh[:], in_=hh[:],
                                     func=mybir.ActivationFunctionType.Relu)
                # hT via PE transpose
                psT = pp.tile([OUT_C, P], dt.float32, tag="psT")
                nc.tensor.transpose(out=psT[:], in_=hh[:], identity=ident[:])
                hT = spool.tile([OUT_C, P], dt.float32, tag="hT")
                nc.vector.tensor_copy(out=hT[:], in_=psT[:])
                # g2 = dinv * (h @ W2); out2 = sigmoid(h @ W3 + b3)
                pst23 = pp.tile([P, OUT_C + 6], dt.float32, tag="pst23")
                nc.tensor.matmul(out=pst23[:, 0:OUT_C], lhsT=hT[:], rhs=W2_t[:],
                                 start=True, stop=True)
                nc.tensor.matmul(out=pst23[:, OUT_C:OUT_C + 6], lhsT=hT[:],
                                 rhs=W3_t[:], start=True, stop=True)
                nc.vector.tensor_scalar(out=g2stage[:, w, :],
                                        in0=pst23[:, 0:OUT_C],
                                        scalar1=dinv_w[:, w:w + 1], scalar2=None,
                                        op0=mybir.AluOpType.mult)
                nc.vector.tensor_tensor(out=o2stage[:, w, :],
                                        in0=pst23[:, OUT_C:OUT_C + 6],
                                        in1=vecs[:, 3 * OUT_C:3 * OUT_C + 6],
                                        op=mybir.AluOpType.add)

            if DEBUG_PHASE == "dense":
                nc.vector.memset(g2stage[:], 0.0)
                nc.vector.memset(o2stage[:], 0.0)
            elif DEBUG_PHASE == "agg":
                nc.vector.memset(o2stage[:], 0.0)
                def epilogue_lite(w, psum):
                    nc.vector.tensor_copy(out=g2stage[:, w, :], in_=psum[:])
                _emit_aggregation(nc, tc, pool, cpool, pp, sched, g1_d, idx_t,
                                  dstoff_t, iota8, epilogue_lite, g1_writes)
            else:
                _emit_aggregation(nc, tc, pool, cpool, pp, sched, g1_d, idx_t,
                                  dstoff_t, iota8, epilogue, g1_writes)

            nc.scalar.activation(out=o2stage[:], in_=o2stage[:],
                                 func=mybir.ActivationFunctionType.Sigmoid)
            nc.sync.dma_start(
                out=g2s_d[:].rearrange("(w p) d -> p w d", p=P),
                in_=g2stage[:])
            nc.sync.dma_start(
                out=out2_d[:].rearrange("(w p) d -> p w d", p=P),
                in_=o2stage[:])
    nc.compile()
    return nc


def build_launch_b(sched):
    nc = bacc.Bacc("TRN2", target_bir_lowering=False, debug=False,
                   num_devices=NCORES)
    dt = mybir.dt
    g2_d = nc.dram_tensor("g2full", [NPAD, P], dt.bfloat16, kind="ExternalInput")
    dinv_w_d = nc.dram_tensor("dinv_w", [P, NWIN], dt.float32, kind="ExternalInput")
    b2_d = nc.dram_tensor("b2vec", [P, OUT_C], dt.float32, kind="ExternalInput")
    ncol = sched["nch_total"] * 8
    idx_d = nc.dram_tensor("idx16", [P, ncol], dt.int16, kind="ExternalInput")
    dstoff_d = nc.dram_tensor("dstoff", [P, sched["nch_total"]], dt.float32,
                              kind="ExternalInput")
    out1_d = nc.dram_tensor("out1s", [NWIN * P, OUT_C], dt.float32,
                            kind="ExternalOutput")

    with tile.TileContext(nc) as tc:
        with (
            tc.tile_pool(name="const", bufs=1) as cpool,
            tc.tile_pool(name="sbuf", bufs=2) as pool,
            tc.tile_pool(name="spool4", bufs=6) as spool,
            tc.tile_pool(name="psum", bufs=2, space="PSUM") as pp,
        ):
            dinv_w = cpool.tile([P, NWIN], dt.float32)
            nc.sync.dma_start(out=dinv_w[:], in_=dinv_w_d[:])
            b2v = cpool.tile([P, OUT_C], dt.float32)
            nc.sync.dma_start(out=b2v[:], in_=b2_d[:])
            idx_t = cpool.tile([P, ncol], dt.int16)
            nc.sync.dma_start(out=idx_t[:], in_=idx_d[:])
            dstoff_t = cpool.tile([P, sched["nch_total"]], dt.float32)
            nc.sync.dma_start(out=dstoff_t[:], in_=dstoff_d[:])
            iota_i = cpool.tile([P, 8, P], dt.int32)
            nc.gpsimd.iota(iota_i[:], pattern=[[0, 8], [1, P]], base=0,
                           channel_multiplier=0)
            iota8 = cpool.tile([P, 8, P], dt.float32)
            nc.vector.tensor_copy(out=iota8[:], in_=iota_i[:])

            o1stage = cpool.tile([P, NWIN, OUT_C], dt.float32)

            def epilogue(w, psum):
                t1 = spool.tile([P, OUT_C], dt.float32, tag="t1")
                nc.vector.tensor_scalar(out=t1[:], in0=psum[:],
                                        scalar1=dinv_w[:, w:w + 1], scalar2=None,
                                        op0=mybir.AluOpType.mult)
                nc.vector.tensor_tensor(out=o1stage[:, w, :], in0=t1[:],
                                        in1=b2v[:], op=mybir.AluOpType.add)

            _emit_aggregation(nc, tc, pool, cpool, pp, sched, g2_d, idx_t,
                              dstoff_t, iota8, epilogue, [])

            nc.sync.dma_start(
                out=out1_d[:].rearrange("(w p) d -> p w d", p=P),
                in_=o1stage[:])
    nc.compile()
    return nc


# --------------------------------------------------------------------------
# entry point
# --------------------------------------------------------------------------

def kernel(x, edge_index, W1, b1, ln_w, ln_b, W2, b2, W3, b3):
    x = np.asarray(x, np.float32)
    edge_index = np.asarray(edge_index)
    W1 = np.asarray(W1, np.float32); b1 = np.asarray(b1, np.float32)
    ln_w = np.asarray(ln_w, np.float32); ln_b = np.asarray(ln_b, np.float32)
    W2 = np.asarray(W2, np.float32); b2 = np.asarray(b2, np.float32)
    W3 = np.asarray(W3, np.float32); b3 = np.asarray(b3, np.float32)

    if TRACE:
        _install_prof_shim()
    del LAST_EXEC_NS[:]
    del LAST_RESULTS[:]

    esrc = edge_index[0].astype(np.int64)
    edst = edge_index[1].astype(np.int64)

    # degrees (incl. self loop) and dinv, host-side (index preprocessing)
    deg = (np.bincount(edst, minlength=N) + 1).astype(np.float32)
    dinv = (1.0 / np.sqrt(deg)).astype(np.float32)

    sched, percore = build_schedule(esrc, edst)

    # host-side input prep
    xT = np.ascontiguousarray(x.T)
    vecs = np.ascontiguousarray(np.tile(np.concatenate(
        [b1, ln_w, ln_b, np.pad(b3, (0, OUT_C - 6))]), (P, 1)).astype(np.float32))
    dinv_pad = np.concatenate([dinv, np.ones(NPAD - N, np.float32)])
    dinv_t = np.ascontiguousarray(dinv_pad.reshape(NTILE, P).T)
    dinv_w_cores = []
    for r in range(NCORES):
        dw = np.ones(NWIN * P, np.float32)
        dw[:SHARD] = dinv[r * SHARD:(r + 1) * SHARD]
        dinv_w_cores.append(np.ascontiguousarray(dw.reshape(NWIN, P).T))

    ncA = build_launch_a(sched)
    in_maps_a = []
    for r in range(NCORES):
        in_maps_a.append({
            "xT": xT, "W1": W1, "W2": W2, "W3": W3, "vecs": vecs,
            "dinv_t": dinv_t, "dinv_w": dinv_w_cores[r],
            "idx16": percore[r]["idx16"], "dstoff": percore[r]["dstoff"],
        })
    resA = run_bass_kernel_spmd(ncA, in_maps_a, core_ids=list(range(NCORES)),
                                trace=TRACE)
    if TRACE:
        LAST_EXEC_NS.append(resA.exec_time_ns)
        LAST_RESULTS.append(resA)

    # assemble full g2 table [NPAD, 128] bf16 (padding cols/rows zero)
    g2full = np.zeros((NPAD, P), ml_dtypes.bfloat16)
    out2 = np.empty((N, 6), np.float32)
    for r in range(NCORES):
        g2s = resA.results[r]["g2s"][:SHARD]           # [6250, 64]
        g2full[r * SHARD:(r + 1) * SHARD, 0:OUT_C] = g2s
        out2[r * SHARD:(r + 1) * SHARD] = resA.results[r]["out2s"][:SHARD]

    b2v = np.ascontiguousarray(np.tile(b2, (P, 1)).astype(np.float32))
    ncB = build_launch_b(sched)
    in_maps_b = []
    for r in range(NCORES):
        in_maps_b.append({
            "g2full": g2full, "dinv_w": dinv_w_cores[r], "b2vec": b2v,
            "idx16": percore[r]["idx16"], "dstoff": percore[r]["dstoff"],
        })
    resB = run_bass_kernel_spmd(ncB, in_maps_b, core_ids=list(range(NCORES)),
                                trace=TRACE)
    if TRACE:
        LAST_EXEC_NS.append(resB.exec_time_ns)
        LAST_RESULTS.append(resB)

    out1 = np.empty((N, OUT_C), np.float32)
    for r in range(NCORES):
        out1[r * SHARD:(r + 1) * SHARD] = resB.results[r]["out1s"][:SHARD]

    return (out1, out2)


# revision 9
# speedup vs baseline: 1.0845x; 1.0279x over previous
"""GCN encoder (2x GCNConv + LN/ReLU + sigmoid head) as a Bass/Trainium2 SPMD kernel.

Strategy (per sharding hint): destinations sharded 6250 nodes/core across 8
cores, edges partitioned by destination; per-core aggregation is done with
dma_gather (source-row gather) + one-hot scatter matmuls accumulating into
PSUM per 128-node destination window.  Two launches: layer 1 (+dense, LN,
heads) and layer 2, with the inter-layer halo exchange (full g2 table) done
on the host between launches.
"""

import os
import sys

for _p in ("/opt/trn_rl_repo", "/opt/pypackages"):
    if _p not in sys.path:
        sys.path.insert(0, _p)

import numpy as np
import ml_dtypes

import concourse.bass as bass
import concourse.mybir as mybir
import concourse.tile as tile
from concourse import bacc
from concourse.bass_utils import run_bass_kernel_spmd
from concourse.masks import make_identity

# ---- problem constants (hardcoded per task contract) ----
N = 50000
IN_C = 128
OUT_C = 64
NCORES = 8
SHARD = N // NCORES          # 6250
P = 128
NWIN = (SHARD + P - 1) // P  # 49 windows/shard; last window 106 nodes
LASTW = SHARD - (NWIN - 1) * P
HALF = 32768                 # int16 gather split
GW = 4                       # windows per gather group
NGRP = (NWIN + GW - 1) // GW # 13 groups (last group 1 window)
NTILE = (N + P - 1) // P     # 391 dense node tiles
NPAD = NTILE * P             # 50048 padded rows of the gather tables
DENSE_CHUNK = 16             # node tiles per dense xT chunk (2048 nodes)
EPS = 1e-5

DEBUG_PHASE = os.environ.get("KDBG", "full")  # dense | agg | full
TRACE = False                # test.py sets True to collect HW exec times
LAST_EXEC_NS = []
LAST_RESULTS = []            # BassKernelResults per launch when TRACE
_PROF_SHIM = False


def _install_prof_shim():
    global _PROF_SHIM
    if _PROF_SHIM:
        return
    try:
        import antenv.axon_hooks  # noqa: F401
    except ImportError:
        import types
        from trn_agent_boot.trn_boot import _ntff_profile_via_ctypes
        hook = _ntff_profile_via_ctypes('/opt/axon/libaxon_pjrt.so')
        mod = types.ModuleType('antenv.axon_hooks')
        mod._hook = hook
        mod.get_axon_ntff_profile_hook = lambda: mod._hook
        mod.set_axon_ntff_profile_hook = lambda h: setattr(mod, '_hook', h)
        sys.modules['antenv.axon_hooks'] = mod
        import antenv
        antenv.axon_hooks = mod
    _PROF_SHIM = True


# --------------------------------------------------------------------------
# host-side schedule construction
# --------------------------------------------------------------------------

def build_schedule(edge_src, edge_dst):
    """Partition edges (+self loops) by destination into a chunk schedule that
    is structurally identical across all 8 cores (counts are maxed over cores,
    shorter cores padded with null edges: idx=0, woff=-1).

    Returns (sched, percore) where
      sched: dict with compile-time constants shared by all cores
      percore[r]: dict with idx16 [128, ncol], dstoff [128, nch] arrays
    """
    src = np.concatenate([edge_src, np.arange(N, dtype=np.int64)])
    dst = np.concatenate([edge_dst, np.arange(N, dtype=np.int64)])

    shard = dst // SHARD
    within = dst % SHARD
    win = within // P
    woff = within % P
    flag = (src >= HALF).astype(np.int64)

    # group edges by (shard, window, flag)
    key = (shard * NWIN + win) * 2 + flag
    order = np.argsort(key, kind="stable")
    skey = key[order]
    ssrc = src[order]
    swoff = woff[order]

    nkeys = NCORES * NWIN * 2
    counts = np.bincount(skey, minlength=nkeys).reshape(NCORES, NWIN, 2)
    starts = np.zeros(nkeys + 1, np.int64)
    np.cumsum(counts.reshape(-1), out=starts[1:])

    # uniform chunk counts: max over cores per (window, flag)
    chunks_rwf = -(-counts // P)                       # ceil
    kch = chunks_rwf.max(axis=0)                       # [NWIN, 2]
    klo, khi = kch[:, 0], kch[:, 1]
    assert klo.min() >= 1 and khi.min() >= 1

    # group layout: for each group, lo chunks of its windows then hi chunks
    grp_windows = [list(range(g * GW, min((g + 1) * GW, NWIN))) for g in range(NGRP)]
    grp_lo = [int(sum(klo[w] for w in ws)) for ws in grp_windows]
    grp_hi = [int(sum(khi[w] for w in ws)) for ws in grp_windows]
    grp_nch = [lo + hi for lo, hi in zip(grp_lo, grp_hi)]
    nch_total = int(sum(grp_nch))

    # global chunk index of each (window, flag) span + per-group call info
    chunk_off = {}
    calls = []   # (grp, flag, chunk_off_global, nchunks)
    off = 0
    for g, ws in enumerate(grp_windows):
        calls.append((g, 0, off, grp_lo[g]))
        for w in ws:
            chunk_off[(w, 0)] = off
            off += int(klo[w])
        calls.append((g, 1, off, grp_hi[g]))
        for w in ws:
            chunk_off[(w, 1)] = off
            off += int(khi[w])
    assert off == nch_total

    # per-core arrays
    percore = []
    for r in range(NCORES):
        idx_all = np.zeros(nch_total * P, np.int64)       # logical edge idx
        dstoff = np.full((P, nch_total), -1.0, np.float32)
        for w in range(NWIN):
            for f in (0, 1):
                k = starts[(r * NWIN + w) * 2 + f]
                e = starts[(r * NWIN + w) * 2 + f + 1]
                cnt = e - k
                co = chunk_off[(w, f)]
                base = HALF if f else 0
                span = idx_all[co * P: co * P + cnt]
                span[:] = ssrc[k:e] - base
                dcol = dstoff[:, co:co + int(kch[w, f])]
                flat = np.full(int(kch[w, f]) * P, -1.0, np.float32)
                flat[:cnt] = swoff[k:e]
                dcol[:] = flat.reshape(-1, P).T
        # wrap idx per gather call: position i -> [i % 16, i // 16], x8 replicate
        ncol = nch_total * (P // 16)
        idx16 = np.zeros((16, ncol), np.int16)
        for (_, _, co, nc_) in calls:
            if nc_ == 0:
                continue
            span = idx_all[co * P:(co + nc_) * P].astype(np.int16)
            blk = span.reshape(-1, 16).T                  # [16, n/16]
            idx16[:, co * 8:(co + nc_) * 8] = blk
        percore.append({
            "idx16": np.ascontiguousarray(np.tile(idx16, (8, 1))),
            "dstoff": dstoff,
        })

    sched = {
        "klo": klo, "khi": khi,
        "grp_windows": grp_windows,
        "grp_lo": grp_lo, "grp_hi": grp_hi, "grp_nch": grp_nch,
        "nch_total": nch_total,
        "chunk_off": chunk_off,
        "calls": calls,
        "max_nch": max(grp_nch),
    }
    return sched, percore


# --------------------------------------------------------------------------
# kernel builders
# --------------------------------------------------------------------------

def _emit_aggregation(nc, tc, pool, gpool, pp, sched, g_dram, idx_t, dstoff_t,
                      iota8, window_epilogue, dep_insts):
    """Shared per-layer aggregation: for each group, 2 gathers + one-hot
    matmuls accumulating per-window PSUM, then window_epilogue(w, psum_tile).
    dep_insts: instructions every gather must wait for (g table writes)."""
    lo_tab = g_dram[0:HALF, :]
    hi_tab = g_dram[HALF:NPAD, :]
    calls = sched["calls"]
    MAXCH = 8   # 1024 idxs/call: SWDGE descriptor-ring capacity
    for g, ws in enumerate(sched["grp_windows"]):
        base_off = sched["chunk_off"][(ws[0], 0)]
        msgs = gpool.tile([P, sched["max_nch"], P], mybir.dt.bfloat16, tag="msgs")
        S_grp = pool.tile([P, sched["max_nch"], P], mybir.dt.bfloat16, tag="Sgrp")
        for (gg, f, co, nc_) in calls:
            if gg != g or nc_ == 0:
                continue
            for s0 in range(0, nc_, MAXCH):
                sco = co + s0
                snc = min(MAXCH, nc_ - s0)
                co0 = sco - base_off                      # slot within group
                gi = nc.gpsimd.dma_gather(
                    out_ap=msgs[:, co0:co0 + snc, :],
                    in_ap=(hi_tab if f else lo_tab),
                    idxs_ap=idx_t[:, sco * 8:(sco + snc) * 8],
                    num_idxs=snc * P,
                    num_idxs_reg=snc * P,
                    elem_size=P,
                )
                deps = (dep_insts.get(1, []) if f else dep_insts.get(0, [])) \
                    if isinstance(dep_insts, dict) else dep_insts
                for d in deps:
                    bass._add_dep_helper(gi.ins, d.ins, True,
                                         "gather after table write")
                # one-hot scatter matrices for this call's chunks, one DVE op
                nc.vector.tensor_tensor(
                    out=S_grp[:, co0:co0 + snc, :],
                    in0=iota8[:, 0:snc, :],
                    in1=dstoff_t[:, sco:sco + snc].broadcast_to([P, snc, P]),
                    op=mybir.AluOpType.is_equal,
                )
        for w in ws:
            nchw = int(sched["klo"][w] + sched["khi"][w])
            spans = [(sched["chunk_off"][(w, 0)], int(sched["klo"][w])),
                     (sched["chunk_off"][(w, 1)], int(sched["khi"][w]))]
            psum = pp.tile([P, OUT_C], mybir.dt.float32, tag="aggps")
            ci = 0
            for (co, k) in spans:
                for c in range(k):
                    lco = co + c - base_off  # slot in group msgs/S tiles
                    nc.tensor.matmul(
                        out=psum[:], lhsT=S_grp[:, lco, :],
                        rhs=msgs[:, lco, 0:OUT_C],
                        start=(ci == 0), stop=(ci == nchw - 1),
                    )
                    ci += 1
            window_epilogue(w, psum)


def build_launch_a(sched):
    nc = bacc.Bacc("TRN2", target_bir_lowering=False, debug=False,
                   num_devices=NCORES)
    dt = mybir.dt
    xT_d = nc.dram_tensor("xT", [IN_C, N], dt.float32, kind="ExternalInput")
    W1_d = nc.dram_tensor("W1", [IN_C, OUT_C], dt.float32, kind="ExternalInput")
    W2_d = nc.dram_tensor("W2", [OUT_C, OUT_C], dt.float32, kind="ExternalInput")
    W3_d = nc.dram_tensor("W3", [OUT_C, 6], dt.float32, kind="ExternalInput")
    vecs_d = nc.dram_tensor("vecs", [P, 4 * OUT_C], dt.float32, kind="ExternalInput")
    dinv_t_d = nc.dram_tensor("dinv_t", [P, NTILE], dt.float32, kind="ExternalInput")
    dinv_w_d = nc.dram_tensor("dinv_w", [P, NWIN], dt.float32, kind="ExternalInput")
    ncol = sched["nch_total"] * 8
    idx_d = nc.dram_tensor("idx16", [P, ncol], dt.int16, kind="ExternalInput")
    dstoff_d = nc.dram_tensor("dstoff", [P, sched["nch_total"]], dt.float32,
                              kind="ExternalInput")
    g1_d = nc.dram_tensor("g1buf", [NPAD, P], dt.bfloat16)      # internal
    g2s_d = nc.dram_tensor("g2s", [NWIN * P, OUT_C], dt.bfloat16,
                           kind="ExternalOutput")
    out2_d = nc.dram_tensor("out2s", [NWIN * P, 6], dt.float32,
                            kind="ExternalOutput")

    with tile.TileContext(nc) as tc:
        with (
            tc.tile_pool(name="const", bufs=1) as cpool,
            tc.tile_pool(name="sbuf", bufs=2) as pool,
            tc.tile_pool(name="gbuf", bufs=3) as gpool,
            tc.tile_pool(name="spool4", bufs=6) as spool,
            tc.tile_pool(name="psum", bufs=2, space="PSUM") as pp,
        ):
            # ---- constants ----
            W1_t = cpool.tile([IN_C, OUT_C], dt.float32)
            nc.sync.dma_start(out=W1_t[:], in_=W1_d[:])
            W2_t = cpool.tile([OUT_C, OUT_C], dt.float32)
            nc.sync.dma_start(out=W2_t[:], in_=W2_d[:])
            W3_t = cpool.tile([OUT_C, 6], dt.float32)
            nc.sync.dma_start(out=W3_t[:], in_=W3_d[:])
            vecs = cpool.tile([P, 4 * OUT_C], dt.float32)
            nc.sync.dma_start(out=vecs[:], in_=vecs_d[:])
            dinv_t = cpool.tile([P, NTILE], dt.float32)
            nc.sync.dma_start(out=dinv_t[:], in_=dinv_t_d[:])
            dinv_w = cpool.tile([P, NWIN], dt.float32)
            nc.sync.dma_start(out=dinv_w[:], in_=dinv_w_d[:])
            idx_t = cpool.tile([P, ncol], dt.int16)
            nc.sync.dma_start(out=idx_t[:], in_=idx_d[:])
            dstoff_t = cpool.tile([P, sched["nch_total"]], dt.float32)
            nc.sync.dma_start(out=dstoff_t[:], in_=dstoff_d[:])
            ident = cpool.tile([P, P], dt.float32)
            make_identity(nc, ident[:])
            eps_t = cpool.tile([P, 1], dt.float32)
            nc.gpsimd.memset(eps_t[:], EPS)
            invD_t = cpool.tile([P, 1], dt.float32)
            nc.gpsimd.memset(invD_t[:], 1.0 / OUT_C)
            iota_i = cpool.tile([P, 8, P], dt.int32)
            nc.gpsimd.iota(iota_i[:], pattern=[[0, 8], [1, P]], base=0,
                           channel_multiplier=0)
            iota8 = cpool.tile([P, 8, P], dt.float32)
            nc.vector.tensor_copy(out=iota8[:], in_=iota_i[:])

            # ---- phase 1: dense g1 = dinv * (x @ W1), bf16, 256B rows ----
            g1_writes = {0: [], 1: []}
            nchunks = (NTILE + DENSE_CHUNK - 1) // DENSE_CHUNK
            for c in range(nchunks):
                t0 = c * DENSE_CHUNK
                nt = min(DENSE_CHUNK, NTILE - t0)
                n0 = t0 * P
                nn = nt * P
                xc = pool.tile([IN_C, DENSE_CHUNK * P], dt.float32, tag="xc")
                nv = min(nn, N - n0)         # valid source columns
                if nv < nn:
                    nc.vector.memset(xc[:, 0:nn], 0.0)
                nc.sync.dma_start(out=xc[:, 0:nv], in_=xT_d[:, n0:n0 + nv])
                stage = pool.tile([P, DENSE_CHUNK, P], dt.bfloat16, tag="stage")
                nc.vector.memset(stage[:], 0.0)
                for q in range(0, nt, 4):
                    qn = min(4, nt - q)
                    ps4 = pp.tile([P, 4, OUT_C], dt.float32, tag="dps")
                    for s in range(qn):
                        nc.tensor.matmul(out=ps4[:, s, :],
                                         lhsT=xc[:, (q + s) * P:(q + s + 1) * P],
                                         rhs=W1_t[:], start=True, stop=True)
                    nc.vector.tensor_tensor(
                        out=stage[:, q:q + qn, 0:OUT_C], in0=ps4[:, 0:qn, :],
                        in1=dinv_t[:, t0 + q:t0 + q + qn]
                            .broadcast_to([P, qn, OUT_C]),
                        op=mybir.AluOpType.mult)
                wi = nc.sync.dma_start(
                    out=g1_d[n0:n0 + nn, :].rearrange("(s p) d -> p s d", p=P),
                    in_=stage[:, 0:nt, :])
                if n0 < HALF:
                    g1_writes[0].append(wi)      # rows < 32768 (lo table)
                if n0 + nn > HALF:
                    g1_writes[1].append(wi)      # rows >= 32768 (hi table)

            # ---- output staging ----
            g2stage = cpool.tile([P, NWIN, OUT_C], dt.bfloat16)
            o2stage = cpool.tile([P, NWIN, 6], dt.float32)

            # ---- phase 2: aggregation + epilogue ----
            def epilogue(w, psum):
                tt = spool.tile([P, OUT_C], dt.float32, tag="tt")
                nc.vector.tensor_scalar(out=tt[:], in0=psum[:],
                                        scalar1=dinv_w[:, w:w + 1], scalar2=None,
                                        op0=mybir.AluOpType.mult)
                nc.vector.tensor_tensor(out=tt[:], in0=tt[:],
                                        in1=vecs[:, 0:OUT_C],
                                        op=mybir.AluOpType.add)
                mu = spool.tile([P, 1], dt.float32, tag="mu")
                nc.vector.tensor_reduce(out=mu[:], in_=tt[:],
                                        axis=mybir.AxisListType.X,
                                        op=mybir.AluOpType.add)
                nc.vector.tensor_scalar(out=mu[:], in0=mu[:], scalar1=1.0 / OUT_C,
                                        scalar2=None, op0=mybir.AluOpType.mult)
                nc.vector.tensor_scalar(out=tt[:], in0=tt[:], scalar1=mu[:],
                                        scalar2=None,
                                        op0=mybir.AluOpType.subtract)
                var = spool.tile([P, 1], dt.float32, tag="var")
                sq = spool.tile([P, OUT_C], dt.float32, tag="sq")
                if os.environ.get("KTTR", "0") == "1":
                    nc.vector.tensor_tensor_reduce(
                        out=sq[:], in0=tt[:], in1=tt[:], scale=1.0, scalar=0.0,
                        op0=mybir.AluOpType.mult, op1=mybir.AluOpType.add,
                        accum_out=var[:])
                else:
                    nc.scalar.activation(out=sq[:], in_=tt[:],
                                         func=mybir.ActivationFunctionType.Square,
                                         accum_out=var[:])
                std = spool.tile([P, 1], dt.float32, tag="std")
                nc.scalar.activation(out=std[:], in_=var[:],
                                     func=mybir.ActivationFunctionType.Sqrt,
                                     scale=invD_t[:], bias=eps_t[:])
                rstd = spool.tile([P, 1], dt.float32, tag="rstd")
                nc.vector.reciprocal(out=rstd[:], in_=std[:])
                hh = spool.tile([P, OUT_C], dt.float32, tag="hh")
                nc.vector.tensor_scalar(out=hh[:], in0=tt[:], scalar1=rstd[:],
                                        scalar2=None, op0=mybir.AluOpType.mult)
                nc.vector.tensor_tensor(out=hh[:], in0=hh[:],
                                        in1=vecs[:, OUT_C:2 * OUT_C],
                                        op=mybir.AluOpType.mult)
                nc.vector.tensor_tensor(out=hh[:], in0=hh[:],
                                        in1=vecs[:, 2 * OUT_C:3 * OUT_C],
                                        op=mybir.AluOpType.add)
                nc.scalar.activation(out=hh[:], in_=hh[:],
                                     func=mybir.ActivationFunctionType.Relu)
                # hT via PE transpose
                psT = pp.tile([OUT_C, P], dt.float32, tag="psT")
                nc.tensor.transpose(out=psT[:], in_=hh[:], identity=ident[:])
                hT = spool.tile([OUT_C, P], dt.float32, tag="hT")
                nc.vector.tensor_copy(out=hT[:], in_=psT[:])
                # g2 = dinv * (h @ W2); out2 = sigmoid(h @ W3 + b3)
                pst23 = pp.tile([P, OUT_C + 6], dt.float32, tag="pst23")
                nc.tensor.matmul(out=pst23[:, 0:OUT_C], lhsT=hT[:], rhs=W2_t[:],
                                 start=True, stop=True)
                nc.tensor.matmul(out=pst23[:, OUT_C:OUT_C + 6], lhsT=hT[:],
                                 rhs=W3_t[:], start=True, stop=True)
                nc.vector.tensor_scalar(out=g2stage[:, w, :],
                                        in0=pst23[:, 0:OUT_C],
                                        scalar1=dinv_w[:, w:w + 1], scalar2=None,
                                        op0=mybir.AluOpType.mult)
                nc.vector.tensor_tensor(out=o2stage[:, w, :],
                                        in0=pst23[:, OUT_C:OUT_C + 6],
                                        in1=vecs[:, 3 * OUT_C:3 * OUT_C + 6],
                                        op=mybir.AluOpType.add)

            if DEBUG_PHASE == "dense":
                nc.vector.memset(g2stage[:], 0.0)
                nc.vector.memset(o2stage[:], 0.0)
            elif DEBUG_PHASE == "agg":
                nc.vector.memset(o2stage[:], 0.0)
                def epilogue_lite(w, psum):
                    nc.vector.tensor_copy(out=g2stage[:, w, :], in_=psum[:])
                _emit_aggregation(nc, tc, pool, gpool, pp, sched, g1_d, idx_t,
                                  dstoff_t, iota8, epilogue_lite, g1_writes)
            else:
                _emit_aggregation(nc, tc, pool, gpool, pp, sched, g1_d, idx_t,
                                  dstoff_t, iota8, epilogue, g1_writes)

            nc.scalar.activation(out=o2stage[:], in_=o2stage[:],
                                 func=mybir.ActivationFunctionType.Sigmoid)
            nc.sync.dma_start(
                out=g2s_d[:].rearrange("(w p) d -> p w d", p=P),
                in_=g2stage[:])
            nc.sync.dma_start(
                out=out2_d[:].rearrange("(w p) d -> p w d", p=P),
                in_=o2stage[:])
    nc.compile()
    return nc


def build_launch_b(sched):
    nc = bacc.Bacc("TRN2", target_bir_lowering=False, debug=False,
                   num_devices=NCORES)
    dt = mybir.dt
    g2_d = nc.dram_tensor("g2full", [NPAD, P], dt.bfloat16, kind="ExternalInput")
    dinv_w_d = nc.dram_tensor("dinv_w", [P, NWIN], dt.float32, kind="ExternalInput")
    b2_d = nc.dram_tensor("b2vec", [P, OUT_C], dt.float32, kind="ExternalInput")
    ncol = sched["nch_total"] * 8
    idx_d = nc.dram_tensor("idx16", [P, ncol], dt.int16, kind="ExternalInput")
    dstoff_d = nc.dram_tensor("dstoff", [P, sched["nch_total"]], dt.float32,
                              kind="ExternalInput")
    out1_d = nc.dram_tensor("out1s", [NWIN * P, OUT_C], dt.float32,
                            kind="ExternalOutput")

    with tile.TileContext(nc) as tc:
        with (
            tc.tile_pool(name="const", bufs=1) as cpool,
            tc.tile_pool(name="sbuf", bufs=2) as pool,
            tc.tile_pool(name="gbuf", bufs=3) as gpool,
            tc.tile_pool(name="spool4", bufs=6) as spool,
            tc.tile_pool(name="psum", bufs=2, space="PSUM") as pp,
        ):
            dinv_w = cpool.tile([P, NWIN], dt.float32)
            nc.sync.dma_start(out=dinv_w[:], in_=dinv_w_d[:])
            b2v = cpool.tile([P, OUT_C], dt.float32)
            nc.sync.dma_start(out=b2v[:], in_=b2_d[:])
            idx_t = cpool.tile([P, ncol], dt.int16)
            nc.sync.dma_start(out=idx_t[:], in_=idx_d[:])
            dstoff_t = cpool.tile([P, sched["nch_total"]], dt.float32)
            nc.sync.dma_start(out=dstoff_t[:], in_=dstoff_d[:])
            iota_i = cpool.tile([P, 8, P], dt.int32)
            nc.gpsimd.iota(iota_i[:], pattern=[[0, 8], [1, P]], base=0,
                           channel_multiplier=0)
            iota8 = cpool.tile([P, 8, P], dt.float32)
            nc.vector.tensor_copy(out=iota8[:], in_=iota_i[:])

            o1stage = cpool.tile([P, NWIN, OUT_C], dt.float32)

            def epilogue(w, psum):
                t1 = spool.tile([P, OUT_C], dt.float32, tag="t1")
                nc.vector.tensor_scalar(out=t1[:], in0=psum[:],
                                        scalar1=dinv_w[:, w:w + 1], scalar2=None,
                                        op0=mybir.AluOpType.mult)
                nc.vector.tensor_tensor(out=o1stage[:, w, :], in0=t1[:],
                                        in1=b2v[:], op=mybir.AluOpType.add)

            _emit_aggregation(nc, tc, pool, gpool, pp, sched, g2_d, idx_t,
                              dstoff_t, iota8, epilogue, [])

            nc.sync.dma_start(
                out=out1_d[:].rearrange("(w p) d -> p w d", p=P),
                in_=o1stage[:])
    nc.compile()
    return nc


# --------------------------------------------------------------------------
# entry point
# --------------------------------------------------------------------------

def kernel(x, edge_index, W1, b1, ln_w, ln_b, W2, b2, W3, b3):
    x = np.asarray(x, np.float32)
    edge_index = np.asarray(edge_index)
    W1 = np.asarray(W1, np.float32); b1 = np.asarray(b1, np.float32)
    ln_w = np.asarray(ln_w, np.float32); ln_b = np.asarray(ln_b, np.float32)
    W2 = np.asarray(W2, np.float32); b2 = np.asarray(b2, np.float32)
    W3 = np.asarray(W3, np.float32); b3 = np.asarray(b3, np.float32)

    if TRACE:
        _install_prof_shim()
    del LAST_EXEC_NS[:]
    del LAST_RESULTS[:]

    esrc = edge_index[0].astype(np.int64)
    edst = edge_index[1].astype(np.int64)

    # degrees (incl. self loop) and dinv, host-side (index preprocessing)
    deg = (np.bincount(edst, minlength=N) + 1).astype(np.float32)
    dinv = (1.0 / np.sqrt(deg)).astype(np.float32)

    sched, percore = build_schedule(esrc, edst)

    # host-side input prep
    xT = np.ascontiguousarray(x.T)
    vecs = np.ascontiguousarray(np.tile(np.concatenate(
        [b1, ln_w, ln_b, np.pad(b3, (0, OUT_C - 6))]), (P, 1)).astype(np.float32))
    dinv_pad = np.concatenate([dinv, np.ones(NPAD - N, np.float32)])
    dinv_t = np.ascontiguousarray(dinv_pad.reshape(NTILE, P).T)
    dinv_w_cores = []
    for r in range(NCORES):
        dw = np.ones(NWIN * P, np.float32)
        dw[:SHARD] = dinv[r * SHARD:(r + 1) * SHARD]
        dinv_w_cores.append(np.ascontiguousarray(dw.reshape(NWIN, P).T))

    ncA = build_launch_a(sched)
    in_maps_a = []
    for r in range(NCORES):
        in_maps_a.append({
            "xT": xT, "W1": W1, "W2": W2, "W3": W3, "vecs": vecs,
            "dinv_t": dinv_t, "dinv_w": dinv_w_cores[r],
            "idx16": percore[r]["idx16"], "dstoff": percore[r]["dstoff"],
        })
    resA = run_bass_kernel_spmd(ncA, in_maps_a, core_ids=list(range(NCORES)),
                                trace=TRACE)
    if TRACE:
        LAST_EXEC_NS.append(resA.exec_time_ns)
        LAST_RESULTS.append(resA)

    # assemble full g2 table [NPAD, 128] bf16 (padding cols/rows zero)
    g2full = np.zeros((NPAD, P), ml_dtypes.bfloat16)
    out2 = np.empty((N, 6), np.float32)
    for r in range(NCORES):
        g2s = resA.results[r]["g2s"][:SHARD]           # [6250, 64]
        g2full[r * SHARD:(r + 1) * SHARD, 0:OUT_C] = g2s
        out2[r * SHARD:(r + 1) * SHARD] = resA.results[r]["out2s"][:SHARD]

    b2v = np.ascontiguousarray(np.tile(b2, (P, 1)).astype(np.float32))
    ncB = build_launch_b(sched)
    in_maps_b = []
    for r in range(NCORES):
        in_maps_b.append({
            "g2full": g2full, "dinv_w": dinv_w_cores[r], "b2vec": b2v,
            "idx16": percore[r]["idx16"], "dstoff": percore[r]["dstoff"],
        })
    resB = run_bass_kernel_spmd(ncB, in_maps_b, core_ids=list(range(NCORES)),
                                trace=TRACE)
    if TRACE:
        LAST_EXEC_NS.append(resB.exec_time_ns)
        LAST_RESULTS.append(resB)

    out1 = np.empty((N, OUT_C), np.float32)
    for r in range(NCORES):
        out1[r * SHARD:(r + 1) * SHARD] = resB.results[r]["out1s"][:SHARD]

    return (out1, out2)


# revision 10
# speedup vs baseline: 1.1163x; 1.0293x over previous
"""GCN encoder (2x GCNConv + LN/ReLU + sigmoid head) as a Bass/Trainium2 SPMD kernel.

Strategy (per sharding hint): destinations sharded 6250 nodes/core across 8
cores, edges partitioned by destination; per-core aggregation is done with
dma_gather (source-row gather) + one-hot scatter matmuls accumulating into
PSUM per 128-node destination window.  Two launches: layer 1 (+dense, LN,
heads) and layer 2, with the inter-layer halo exchange (full g2 table) done
on the host between launches.
"""

import os
import sys

for _p in ("/opt/trn_rl_repo", "/opt/pypackages"):
    if _p not in sys.path:
        sys.path.insert(0, _p)

import numpy as np
import ml_dtypes

import concourse.bass as bass
import concourse.mybir as mybir
import concourse.tile as tile
from concourse import bacc
from concourse.bass_utils import run_bass_kernel_spmd
from concourse.masks import make_identity

# ---- problem constants (hardcoded per task contract) ----
N = 50000
IN_C = 128
OUT_C = 64
NCORES = 8
SHARD = N // NCORES          # 6250
P = 128
NWIN = (SHARD + P - 1) // P  # 49 windows/shard; last window 106 nodes
LASTW = SHARD - (NWIN - 1) * P
HALF = 32768                 # int16 gather split
GW = 4                       # windows per gather group
NGRP = (NWIN + GW - 1) // GW # 13 groups (last group 1 window)
NTILE = (N + P - 1) // P     # 391 dense node tiles
NPAD = NTILE * P             # 50048 padded rows of the gather tables
DENSE_CHUNK = 16             # node tiles per dense xT chunk (2048 nodes)
EPS = 1e-5

DEBUG_PHASE = os.environ.get("KDBG", "full")  # dense | agg | full
TRACE = False                # test.py sets True to collect HW exec times
LAST_EXEC_NS = []
LAST_RESULTS = []            # BassKernelResults per launch when TRACE
_PROF_SHIM = False


def _install_prof_shim():
    global _PROF_SHIM
    if _PROF_SHIM:
        return
    try:
        import antenv.axon_hooks  # noqa: F401
    except ImportError:
        import types
        from trn_agent_boot.trn_boot import _ntff_profile_via_ctypes
        hook = _ntff_profile_via_ctypes('/opt/axon/libaxon_pjrt.so')
        mod = types.ModuleType('antenv.axon_hooks')
        mod._hook = hook
        mod.get_axon_ntff_profile_hook = lambda: mod._hook
        mod.set_axon_ntff_profile_hook = lambda h: setattr(mod, '_hook', h)
        sys.modules['antenv.axon_hooks'] = mod
        import antenv
        antenv.axon_hooks = mod
    _PROF_SHIM = True


# --------------------------------------------------------------------------
# host-side schedule construction
# --------------------------------------------------------------------------

def build_schedule(edge_src, edge_dst):
    """Partition edges (+self loops) by destination into a chunk schedule that
    is structurally identical across all 8 cores (counts are maxed over cores,
    shorter cores padded with null edges: idx=0, woff=-1).

    Returns (sched, percore) where
      sched: dict with compile-time constants shared by all cores
      percore[r]: dict with idx16 [128, ncol], dstoff [128, nch] arrays
    """
    src = np.concatenate([edge_src, np.arange(N, dtype=np.int64)])
    dst = np.concatenate([edge_dst, np.arange(N, dtype=np.int64)])

    shard = dst // SHARD
    within = dst % SHARD
    win = within // P
    woff = within % P
    flag = (src >= HALF).astype(np.int64)

    # group edges by (shard, window, flag)
    key = (shard * NWIN + win) * 2 + flag
    order = np.argsort(key, kind="stable")
    skey = key[order]
    ssrc = src[order]
    swoff = woff[order]

    nkeys = NCORES * NWIN * 2
    counts = np.bincount(skey, minlength=nkeys).reshape(NCORES, NWIN, 2)
    starts = np.zeros(nkeys + 1, np.int64)
    np.cumsum(counts.reshape(-1), out=starts[1:])

    # uniform chunk counts: max over cores per (window, flag)
    chunks_rwf = -(-counts // P)                       # ceil
    kch = chunks_rwf.max(axis=0)                       # [NWIN, 2]
    klo, khi = kch[:, 0], kch[:, 1]
    assert klo.min() >= 1 and khi.min() >= 1

    # group layout: for each group, lo chunks of its windows then hi chunks
    grp_windows = [list(range(g * GW, min((g + 1) * GW, NWIN))) for g in range(NGRP)]
    grp_lo = [int(sum(klo[w] for w in ws)) for ws in grp_windows]
    grp_hi = [int(sum(khi[w] for w in ws)) for ws in grp_windows]
    grp_nch = [lo + hi for lo, hi in zip(grp_lo, grp_hi)]
    nch_total = int(sum(grp_nch))

    # global chunk index of each (window, flag) span + per-group call info
    chunk_off = {}
    calls = []   # (grp, flag, chunk_off_global, nchunks)
    off = 0
    for g, ws in enumerate(grp_windows):
        calls.append((g, 0, off, grp_lo[g]))
        for w in ws:
            chunk_off[(w, 0)] = off
            off += int(klo[w])
        calls.append((g, 1, off, grp_hi[g]))
        for w in ws:
            chunk_off[(w, 1)] = off
            off += int(khi[w])
    assert off == nch_total

    # per-core arrays
    percore = []
    for r in range(NCORES):
        idx_all = np.zeros(nch_total * P, np.int64)       # logical edge idx
        dstoff = np.full((P, nch_total), -1.0, np.float32)
        for w in range(NWIN):
            for f in (0, 1):
                k = starts[(r * NWIN + w) * 2 + f]
                e = starts[(r * NWIN + w) * 2 + f + 1]
                cnt = e - k
                co = chunk_off[(w, f)]
                base = HALF if f else 0
                span = idx_all[co * P: co * P + cnt]
                span[:] = ssrc[k:e] - base
                dcol = dstoff[:, co:co + int(kch[w, f])]
                flat = np.full(int(kch[w, f]) * P, -1.0, np.float32)
                flat[:cnt] = swoff[k:e]
                dcol[:] = flat.reshape(-1, P).T
        # wrap idx per gather call: position i -> [i % 16, i // 16], x8 replicate
        ncol = nch_total * (P // 16)
        idx16 = np.zeros((16, ncol), np.int16)
        for (_, _, co, nc_) in calls:
            if nc_ == 0:
                continue
            span = idx_all[co * P:(co + nc_) * P].astype(np.int16)
            blk = span.reshape(-1, 16).T                  # [16, n/16]
            idx16[:, co * 8:(co + nc_) * 8] = blk
        percore.append({
            "idx16": np.ascontiguousarray(np.tile(idx16, (8, 1))),
            "dstoff": dstoff,
        })

    sched = {
        "klo": klo, "khi": khi,
        "grp_windows": grp_windows,
        "grp_lo": grp_lo, "grp_hi": grp_hi, "grp_nch": grp_nch,
        "nch_total": nch_total,
        "chunk_off": chunk_off,
        "calls": calls,
        "max_nch": max(grp_nch),
    }
    return sched, percore


# --------------------------------------------------------------------------
# kernel builders
# --------------------------------------------------------------------------

def _emit_aggregation(nc, tc, pool, gpool, pp, sched, g_dram, idx_t, dstoff_t,
                      iota8, window_epilogue, dep_insts):
    """Shared per-layer aggregation: for each group, 2 gathers + one-hot
    matmuls accumulating per-window PSUM, then window_epilogue(w, psum_tile).
    dep_insts: instructions every gather must wait for (g table writes)."""
    lo_tab = g_dram[0:HALF, :]
    hi_tab = g_dram[HALF:NPAD, :]
    calls = sched["calls"]
    MAXCH = 8   # 1024 idxs/call: SWDGE descriptor-ring capacity
    for g, ws in enumerate(sched["grp_windows"]):
        base_off = sched["chunk_off"][(ws[0], 0)]
        msgs = gpool.tile([P, sched["max_nch"], P], mybir.dt.bfloat16, tag="msgs")
        S_grp = pool.tile([P, sched["max_nch"], P], mybir.dt.bfloat16, tag="Sgrp")
        for (gg, f, co, nc_) in calls:
            if gg != g or nc_ == 0:
                continue
            for s0 in range(0, nc_, MAXCH):
                sco = co + s0
                snc = min(MAXCH, nc_ - s0)
                co0 = sco - base_off                      # slot within group
                gi = nc.gpsimd.dma_gather(
                    out_ap=msgs[:, co0:co0 + snc, :],
                    in_ap=(hi_tab if f else lo_tab),
                    idxs_ap=idx_t[:, sco * 8:(sco + snc) * 8],
                    num_idxs=snc * P,
                    num_idxs_reg=snc * P,
                    elem_size=P,
                )
                deps = (dep_insts.get(1, []) if f else dep_insts.get(0, [])) \
                    if isinstance(dep_insts, dict) else dep_insts
                for d in deps:
                    bass._add_dep_helper(gi.ins, d.ins, True,
                                         "gather after table write")
                # one-hot scatter matrices for this call's chunks, one DVE op
                nc.vector.tensor_tensor(
                    out=S_grp[:, co0:co0 + snc, :],
                    in0=iota8[:, 0:snc, :],
                    in1=dstoff_t[:, sco:sco + snc].broadcast_to([P, snc, P]),
                    op=mybir.AluOpType.is_equal,
                )
        for w in ws:
            nchw = int(sched["klo"][w] + sched["khi"][w])
            spans = [(sched["chunk_off"][(w, 0)], int(sched["klo"][w])),
                     (sched["chunk_off"][(w, 1)], int(sched["khi"][w]))]
            psum = pp.tile([P, OUT_C], mybir.dt.float32, tag="aggps")
            ci = 0
            for (co, k) in spans:
                for c in range(k):
                    lco = co + c - base_off  # slot in group msgs/S tiles
                    nc.tensor.matmul(
                        out=psum[:], lhsT=S_grp[:, lco, :],
                        rhs=msgs[:, lco, 0:OUT_C],
                        start=(ci == 0), stop=(ci == nchw - 1),
                    )
                    ci += 1
            window_epilogue(w, psum)


def build_launch_a(sched):
    nc = bacc.Bacc("TRN2", target_bir_lowering=False, debug=False,
                   num_devices=NCORES)
    dt = mybir.dt
    xT_d = nc.dram_tensor("xT", [IN_C, N], dt.bfloat16, kind="ExternalInput")
    W1_d = nc.dram_tensor("W1", [IN_C, OUT_C], dt.bfloat16, kind="ExternalInput")
    W2_d = nc.dram_tensor("W2", [OUT_C, OUT_C], dt.float32, kind="ExternalInput")
    W3_d = nc.dram_tensor("W3", [OUT_C, 6], dt.float32, kind="ExternalInput")
    vecs_d = nc.dram_tensor("vecs", [P, 4 * OUT_C], dt.float32, kind="ExternalInput")
    dinv_t_d = nc.dram_tensor("dinv_t", [P, NTILE], dt.float32, kind="ExternalInput")
    dinv_w_d = nc.dram_tensor("dinv_w", [P, NWIN], dt.float32, kind="ExternalInput")
    ncol = sched["nch_total"] * 8
    idx_d = nc.dram_tensor("idx16", [P, ncol], dt.int16, kind="ExternalInput")
    dstoff_d = nc.dram_tensor("dstoff", [P, sched["nch_total"]], dt.float32,
                              kind="ExternalInput")
    g1_d = nc.dram_tensor("g1buf", [NPAD, P], dt.bfloat16)      # internal
    g2s_d = nc.dram_tensor("g2s", [NWIN * P, OUT_C], dt.bfloat16,
                           kind="ExternalOutput")
    out2_d = nc.dram_tensor("out2s", [NWIN * P, 6], dt.float32,
                            kind="ExternalOutput")

    with tile.TileContext(nc) as tc:
        with (
            tc.tile_pool(name="const", bufs=1) as cpool,
            tc.tile_pool(name="sbuf", bufs=2) as pool,
            tc.tile_pool(name="gbuf", bufs=3) as gpool,
            tc.tile_pool(name="spool4", bufs=6) as spool,
            tc.tile_pool(name="psum", bufs=2, space="PSUM") as pp,
        ):
            # ---- constants ----
            W1_t = cpool.tile([IN_C, OUT_C], dt.bfloat16)
            nc.sync.dma_start(out=W1_t[:], in_=W1_d[:])
            W2_t = cpool.tile([OUT_C, OUT_C], dt.float32)
            nc.sync.dma_start(out=W2_t[:], in_=W2_d[:])
            W3_t = cpool.tile([OUT_C, 6], dt.float32)
            nc.sync.dma_start(out=W3_t[:], in_=W3_d[:])
            vecs = cpool.tile([P, 4 * OUT_C], dt.float32)
            nc.sync.dma_start(out=vecs[:], in_=vecs_d[:])
            dinv_t = cpool.tile([P, NTILE], dt.float32)
            nc.sync.dma_start(out=dinv_t[:], in_=dinv_t_d[:])
            dinv_w = cpool.tile([P, NWIN], dt.float32)
            nc.sync.dma_start(out=dinv_w[:], in_=dinv_w_d[:])
            idx_t = cpool.tile([P, ncol], dt.int16)
            nc.sync.dma_start(out=idx_t[:], in_=idx_d[:])
            dstoff_t = cpool.tile([P, sched["nch_total"]], dt.float32)
            nc.sync.dma_start(out=dstoff_t[:], in_=dstoff_d[:])
            ident = cpool.tile([P, P], dt.float32)
            make_identity(nc, ident[:])
            eps_t = cpool.tile([P, 1], dt.float32)
            nc.gpsimd.memset(eps_t[:], EPS)
            invD_t = cpool.tile([P, 1], dt.float32)
            nc.gpsimd.memset(invD_t[:], 1.0 / OUT_C)
            iota_i = cpool.tile([P, 8, P], dt.int32)
            nc.gpsimd.iota(iota_i[:], pattern=[[0, 8], [1, P]], base=0,
                           channel_multiplier=0)
            iota8 = cpool.tile([P, 8, P], dt.float32)
            nc.vector.tensor_copy(out=iota8[:], in_=iota_i[:])

            # ---- phase 1: dense g1 = dinv * (x @ W1), bf16, 256B rows ----
            g1_writes = {0: [], 1: []}
            nchunks = (NTILE + DENSE_CHUNK - 1) // DENSE_CHUNK
            for c in range(nchunks):
                t0 = c * DENSE_CHUNK
                nt = min(DENSE_CHUNK, NTILE - t0)
                n0 = t0 * P
                nn = nt * P
                xc = pool.tile([IN_C, DENSE_CHUNK * P], dt.bfloat16, tag="xc")
                nv = min(nn, N - n0)         # valid source columns
                if nv < nn:
                    nc.vector.memset(xc[:, 0:nn], 0.0)
                nc.sync.dma_start(out=xc[:, 0:nv], in_=xT_d[:, n0:n0 + nv])
                stage = pool.tile([P, DENSE_CHUNK, P], dt.bfloat16, tag="stage")
                nc.vector.memset(stage[:], 0.0)
                for q in range(0, nt, 4):
                    qn = min(4, nt - q)
                    ps4 = pp.tile([P, 4, OUT_C], dt.float32, tag="dps")
                    for s in range(qn):
                        nc.tensor.matmul(out=ps4[:, s, :],
                                         lhsT=xc[:, (q + s) * P:(q + s + 1) * P],
                                         rhs=W1_t[:], start=True, stop=True)
                    nc.vector.tensor_tensor(
                        out=stage[:, q:q + qn, 0:OUT_C], in0=ps4[:, 0:qn, :],
                        in1=dinv_t[:, t0 + q:t0 + q + qn]
                            .broadcast_to([P, qn, OUT_C]),
                        op=mybir.AluOpType.mult)
                wi = nc.sync.dma_start(
                    out=g1_d[n0:n0 + nn, :].rearrange("(s p) d -> p s d", p=P),
                    in_=stage[:, 0:nt, :])
                if n0 < HALF:
                    g1_writes[0].append(wi)      # rows < 32768 (lo table)
                if n0 + nn > HALF:
                    g1_writes[1].append(wi)      # rows >= 32768 (hi table)

            # ---- output staging ----
            g2stage = cpool.tile([P, NWIN, OUT_C], dt.bfloat16)
            o2stage = cpool.tile([P, NWIN, 6], dt.float32)

            # ---- phase 2: aggregation + epilogue ----
            def epilogue(w, psum):
                tt = spool.tile([P, OUT_C], dt.float32, tag="tt")
                nc.vector.tensor_scalar(out=tt[:], in0=psum[:],
                                        scalar1=dinv_w[:, w:w + 1], scalar2=None,
                                        op0=mybir.AluOpType.mult)
                nc.vector.tensor_tensor(out=tt[:], in0=tt[:],
                                        in1=vecs[:, 0:OUT_C],
                                        op=mybir.AluOpType.add)
                mu = spool.tile([P, 1], dt.float32, tag="mu")
                nc.vector.tensor_reduce(out=mu[:], in_=tt[:],
                                        axis=mybir.AxisListType.X,
                                        op=mybir.AluOpType.add)
                nc.vector.tensor_scalar(out=mu[:], in0=mu[:], scalar1=1.0 / OUT_C,
                                        scalar2=None, op0=mybir.AluOpType.mult)
                nc.vector.tensor_scalar(out=tt[:], in0=tt[:], scalar1=mu[:],
                                        scalar2=None,
                                        op0=mybir.AluOpType.subtract)
                var = spool.tile([P, 1], dt.float32, tag="var")
                sq = spool.tile([P, OUT_C], dt.float32, tag="sq")
                if os.environ.get("KTTR", "0") == "1":
                    nc.vector.tensor_tensor_reduce(
                        out=sq[:], in0=tt[:], in1=tt[:], scale=1.0, scalar=0.0,
                        op0=mybir.AluOpType.mult, op1=mybir.AluOpType.add,
                        accum_out=var[:])
                else:
                    nc.scalar.activation(out=sq[:], in_=tt[:],
                                         func=mybir.ActivationFunctionType.Square,
                                         accum_out=var[:])
                std = spool.tile([P, 1], dt.float32, tag="std")
                nc.scalar.activation(out=std[:], in_=var[:],
                                     func=mybir.ActivationFunctionType.Sqrt,
                                     scale=invD_t[:], bias=eps_t[:])
                rstd = spool.tile([P, 1], dt.float32, tag="rstd")
                nc.vector.reciprocal(out=rstd[:], in_=std[:])
                hh = spool.tile([P, OUT_C], dt.float32, tag="hh")
                nc.vector.tensor_scalar(out=hh[:], in0=tt[:], scalar1=rstd[:],
                                        scalar2=None, op0=mybir.AluOpType.mult)
                nc.vector.tensor_tensor(out=hh[:], in0=hh[:],
                                        in1=vecs[:, OUT_C:2 * OUT_C],
                                        op=mybir.AluOpType.mult)
                nc.vector.tensor_tensor(out=hh[:], in0=hh[:],
                                        in1=vecs[:, 2 * OUT_C:3 * OUT_C],
                                        op=mybir.AluOpType.add)
                nc.scalar.activation(out=hh[:], in_=hh[:],
                                     func=mybir.ActivationFunctionType.Relu)
                # hT via PE transpose
                psT = pp.tile([OUT_C, P], dt.float32, tag="psT")
                nc.tensor.transpose(out=psT[:], in_=hh[:], identity=ident[:])
                hT = spool.tile([OUT_C, P], dt.float32, tag="hT")
                nc.vector.tensor_copy(out=hT[:], in_=psT[:])
                # g2 = dinv * (h @ W2); out2 = sigmoid(h @ W3 + b3)
                pst23 = pp.tile([P, OUT_C + 6], dt.float32, tag="pst23")
                nc.tensor.matmul(out=pst23[:, 0:OUT_C], lhsT=hT[:], rhs=W2_t[:],
                                 start=True, stop=True)
                nc.tensor.matmul(out=pst23[:, OUT_C:OUT_C + 6], lhsT=hT[:],
                                 rhs=W3_t[:], start=True, stop=True)
                nc.vector.tensor_scalar(out=g2stage[:, w, :],
                                        in0=pst23[:, 0:OUT_C],
                                        scalar1=dinv_w[:, w:w + 1], scalar2=None,
                                        op0=mybir.AluOpType.mult)
                nc.vector.tensor_tensor(out=o2stage[:, w, :],
                                        in0=pst23[:, OUT_C:OUT_C + 6],
                                        in1=vecs[:, 3 * OUT_C:3 * OUT_C + 6],
                                        op=mybir.AluOpType.add)

            if DEBUG_PHASE == "dense":
                nc.vector.memset(g2stage[:], 0.0)
                nc.vector.memset(o2stage[:], 0.0)
            elif DEBUG_PHASE == "agg":
                nc.vector.memset(o2stage[:], 0.0)
                def epilogue_lite(w, psum):
                    nc.vector.tensor_copy(out=g2stage[:, w, :], in_=psum[:])
                _emit_aggregation(nc, tc, pool, gpool, pp, sched, g1_d, idx_t,
                                  dstoff_t, iota8, epilogue_lite, g1_writes)
            else:
                _emit_aggregation(nc, tc, pool, gpool, pp, sched, g1_d, idx_t,
                                  dstoff_t, iota8, epilogue, g1_writes)

            nc.scalar.activation(out=o2stage[:], in_=o2stage[:],
                                 func=mybir.ActivationFunctionType.Sigmoid)
            nc.sync.dma_start(
                out=g2s_d[:].rearrange("(w p) d -> p w d", p=P),
                in_=g2stage[:])
            nc.sync.dma_start(
                out=out2_d[:].rearrange("(w p) d -> p w d", p=P),
                in_=o2stage[:])
    nc.compile()
    return nc


def build_launch_b(sched):
    nc = bacc.Bacc("TRN2", target_bir_lowering=False, debug=False,
                   num_devices=NCORES)
    dt = mybir.dt
    g2_d = nc.dram_tensor("g2full", [NPAD, P], dt.bfloat16, kind="ExternalInput")
    dinv_w_d = nc.dram_tensor("dinv_w", [P, NWIN], dt.float32, kind="ExternalInput")
    b2_d = nc.dram_tensor("b2vec", [P, OUT_C], dt.float32, kind="ExternalInput")
    ncol = sched["nch_total"] * 8
    idx_d = nc.dram_tensor("idx16", [P, ncol], dt.int16, kind="ExternalInput")
    dstoff_d = nc.dram_tensor("dstoff", [P, sched["nch_total"]], dt.float32,
                              kind="ExternalInput")
    out1_d = nc.dram_tensor("out1s", [NWIN * P, OUT_C], dt.float32,
                            kind="ExternalOutput")

    with tile.TileContext(nc) as tc:
        with (
            tc.tile_pool(name="const", bufs=1) as cpool,
            tc.tile_pool(name="sbuf", bufs=2) as pool,
            tc.tile_pool(name="gbuf", bufs=3) as gpool,
            tc.tile_pool(name="spool4", bufs=6) as spool,
            tc.tile_pool(name="psum", bufs=2, space="PSUM") as pp,
        ):
            dinv_w = cpool.tile([P, NWIN], dt.float32)
            nc.sync.dma_start(out=dinv_w[:], in_=dinv_w_d[:])
            b2v = cpool.tile([P, OUT_C], dt.float32)
            nc.sync.dma_start(out=b2v[:], in_=b2_d[:])
            idx_t = cpool.tile([P, ncol], dt.int16)
            nc.sync.dma_start(out=idx_t[:], in_=idx_d[:])
            dstoff_t = cpool.tile([P, sched["nch_total"]], dt.float32)
            nc.sync.dma_start(out=dstoff_t[:], in_=dstoff_d[:])
            iota_i = cpool.tile([P, 8, P], dt.int32)
            nc.gpsimd.iota(iota_i[:], pattern=[[0, 8], [1, P]], base=0,
                           channel_multiplier=0)
            iota8 = cpool.tile([P, 8, P], dt.float32)
            nc.vector.tensor_copy(out=iota8[:], in_=iota_i[:])

            o1stage = cpool.tile([P, NWIN, OUT_C], dt.float32)

            def epilogue(w, psum):
                t1 = spool.tile([P, OUT_C], dt.float32, tag="t1")
                nc.vector.tensor_scalar(out=t1[:], in0=psum[:],
                                        scalar1=dinv_w[:, w:w + 1], scalar2=None,
                                        op0=mybir.AluOpType.mult)
                nc.vector.tensor_tensor(out=o1stage[:, w, :], in0=t1[:],
                                        in1=b2v[:], op=mybir.AluOpType.add)

            _emit_aggregation(nc, tc, pool, gpool, pp, sched, g2_d, idx_t,
                              dstoff_t, iota8, epilogue, [])

            nc.sync.dma_start(
                out=out1_d[:].rearrange("(w p) d -> p w d", p=P),
                in_=o1stage[:])
    nc.compile()
    return nc


# --------------------------------------------------------------------------
# entry point
# --------------------------------------------------------------------------

def kernel(x, edge_index, W1, b1, ln_w, ln_b, W2, b2, W3, b3):
    x = np.asarray(x, np.float32)
    edge_index = np.asarray(edge_index)
    W1 = np.asarray(W1, np.float32); b1 = np.asarray(b1, np.float32)
    ln_w = np.asarray(ln_w, np.float32); ln_b = np.asarray(ln_b, np.float32)
    W2 = np.asarray(W2, np.float32); b2 = np.asarray(b2, np.float32)
    W3 = np.asarray(W3, np.float32); b3 = np.asarray(b3, np.float32)

    if TRACE:
        _install_prof_shim()
    del LAST_EXEC_NS[:]
    del LAST_RESULTS[:]

    esrc = edge_index[0].astype(np.int64)
    edst = edge_index[1].astype(np.int64)

    # degrees (incl. self loop) and dinv, host-side (index preprocessing)
    deg = (np.bincount(edst, minlength=N) + 1).astype(np.float32)
    dinv = (1.0 / np.sqrt(deg)).astype(np.float32)

    sched, percore = build_schedule(esrc, edst)

    # host-side input prep
    xT = np.ascontiguousarray(x.T.astype(ml_dtypes.bfloat16))
    vecs = np.ascontiguousarray(np.tile(np.concatenate(
        [b1, ln_w, ln_b, np.pad(b3, (0, OUT_C - 6))]), (P, 1)).astype(np.float32))
    dinv_pad = np.concatenate([dinv, np.ones(NPAD - N, np.float32)])
    dinv_t = np.ascontiguousarray(dinv_pad.reshape(NTILE, P).T)
    dinv_w_cores = []
    for r in range(NCORES):
        dw = np.ones(NWIN * P, np.float32)
        dw[:SHARD] = dinv[r * SHARD:(r + 1) * SHARD]
        dinv_w_cores.append(np.ascontiguousarray(dw.reshape(NWIN, P).T))

    ncA = build_launch_a(sched)
    in_maps_a = []
    for r in range(NCORES):
        in_maps_a.append({
            "xT": xT, "W1": W1.astype(ml_dtypes.bfloat16),
            "W2": W2, "W3": W3, "vecs": vecs,
            "dinv_t": dinv_t, "dinv_w": dinv_w_cores[r],
            "idx16": percore[r]["idx16"], "dstoff": percore[r]["dstoff"],
        })
    resA = run_bass_kernel_spmd(ncA, in_maps_a, core_ids=list(range(NCORES)),
                                trace=TRACE)
    if TRACE:
        LAST_EXEC_NS.append(resA.exec_time_ns)
        LAST_RESULTS.append(resA)

    # assemble full g2 table [NPAD, 128] bf16 (padding cols/rows zero)
    g2full = np.zeros((NPAD, P), ml_dtypes.bfloat16)
    out2 = np.empty((N, 6), np.float32)
    for r in range(NCORES):
        g2s = resA.results[r]["g2s"][:SHARD]           # [6250, 64]
        g2full[r * SHARD:(r + 1) * SHARD, 0:OUT_C] = g2s
        out2[r * SHARD:(r + 1) * SHARD] = resA.results[r]["out2s"][:SHARD]

    b2v = np.ascontiguousarray(np.tile(b2, (P, 1)).astype(np.float32))
    ncB = build_launch_b(sched)
    in_maps_b = []
    for r in range(NCORES):
        in_maps_b.append({
            "g2full": g2full, "dinv_w": dinv_w_cores[r], "b2vec": b2v,
            "idx16": percore[r]["idx16"], "dstoff": percore[r]["dstoff"],
        })
    resB = run_bass_kernel_spmd(ncB, in_maps_b, core_ids=list(range(NCORES)),
                                trace=TRACE)
    if TRACE:
        LAST_EXEC_NS.append(resB.exec_time_ns)
        LAST_RESULTS.append(resB)

    out1 = np.empty((N, OUT_C), np.float32)
    for r in range(NCORES):
        out1[r * SHARD:(r + 1) * SHARD] = resB.results[r]["out1s"][:SHARD]

    return (out1, out2)
